# revision 19
# baseline (speedup 1.0000x reference)
"""Trainium2 distributed kernel for nn_ARDecoder (2x1024 tokens, D=1024,
H=16/KV=4 GQA, DFF=4096, V=32000, P=256, 4 layers).

Strategy: data-parallel over the 2048 (batch*seq) rows -- 256 rows per core.
Weights are replicated (bf16), activations stay SBUF-resident. Causal
attention uses a per-batch AllGather of K^T and V (replica groups
[[0..3],[4..7]]). The logits GEMM is vocab-sharded: h@patch_W.T is
all-gathered (tiny) and each core computes its 4000-column slice of E.
Host does the embedding gather E[x], norm-weight folding, transposes to
K-major weight layouts, and the final concat over vocab shards.
"""

import os
import numpy as np
import ml_dtypes

import concourse.bass as bass
import concourse.bacc as bacc
import concourse.mybir as mybir
import concourse.tile as tile
from concourse.bass_utils import run_bass_kernel_spmd
from concourse.masks import make_identity

BF16 = mybir.dt.bfloat16
F32 = mybir.dt.float32
AF = mybir.ActivationFunctionType

N_CORES = 8
CORE_IDS = list(range(N_CORES))
B, T, D, H, KV, HD, DFF, V, P, DLAT, L = 2, 1024, 1024, 16, 4, 64, 4096, 32000, 256, 512, 4
EPS = 1e-6
R = 256            # rows per core
RT = 2             # row tiles of 128
DKT = D // 128     # 8 k-tiles over D
VSH = V // N_CORES # 4000 vocab columns per core
NKT = 8            # key tiles of 128 within a batch

_cache = {}


def build(dbg=False):
    key = ("nc", dbg)
    if key in _cache:
        return _cache[key]
    nc = bacc.Bacc("TRN2", target_bir_lowering=False, debug=False,
                   num_devices=N_CORES)
    dbg_t = {}
    if dbg:
        for name, shape, dt in [
            ("dbg_h0", [128, RT, D], F32), ("dbg_h1", [128, RT, D], F32),
            ("dbg_h2", [128, RT, D], F32), ("dbg_h3", [128, RT, D], F32),
            ("dbg_h4", [128, RT, D], F32),
            ("dbg_qT", [128, DKT, R], BF16), ("dbg_kT", [128, KV, T], BF16),
            ("dbg_v65", [128, NKT, KV, HD + 1], BF16),
            ("dbg_oT", [128, 8, R], BF16), ("dbg_hp", [128, 2, R], BF16),
        ]:
            dbg_t[name] = nc.dram_tensor(name, shape, dt, kind="ExternalOutput")

    # ---- parameters (per-core inputs) ----
    ex_augT = nc.dram_tensor("ex_augT", [384, R], BF16, kind="ExternalInput")
    w_emb = nc.dram_tensor("w_emb", [384, D], BF16, kind="ExternalInput")
    wqT = nc.dram_tensor("wqT", [L, D, D], BF16, kind="ExternalInput")
    wkT = nc.dram_tensor("wkT", [L, D, KV * HD], BF16, kind="ExternalInput")
    wvT = nc.dram_tensor("wvT", [L, D, KV * HD], BF16, kind="ExternalInput")
    woT = nc.dram_tensor("woT", [L, D, D], BF16, kind="ExternalInput")
    wgT = nc.dram_tensor("wgT", [L, D, DFF], BF16, kind="ExternalInput")
    wuT = nc.dram_tensor("wuT", [L, D, DFF], BF16, kind="ExternalInput")
    wdT = nc.dram_tensor("wdT", [L, DFF, D], BF16, kind="ExternalInput")
    n1rep = nc.dram_tensor("n1rep", [L, 128, D], F32, kind="ExternalInput")
    mask2 = nc.dram_tensor("mask2", [NKT, 128, 2, R], BF16, kind="ExternalInput")
    patchT = nc.dram_tensor("patchT", [D, P], BF16, kind="ExternalInput")
    ecT = nc.dram_tensor("ecT", [P, VSH], BF16, kind="ExternalInput")
    out = nc.dram_tensor("logits", [B * T, VSH], BF16, kind="ExternalOutput")

    # ---- internal DRAM (collective bounce buffers) ----
    k_in, k_g, v_in, v_g = [], [], [], []
    for l in range(L):
        k_in.append(nc.dram_tensor(f"k_in{l}", [64, KV, R], BF16))
        k_g.append(nc.dram_tensor(f"k_g{l}", [4, 64, KV, R], BF16))
        v_in.append(nc.dram_tensor(f"v_in{l}", [128, RT, KV, HD], BF16))
        v_g.append(nc.dram_tensor(f"v_g{l}", [4, 128, RT, KV, HD], BF16))
    hp_in = nc.dram_tensor("hp_in", [128, 2, R], BF16)
    hp_g = nc.dram_tensor("hp_g", [8, 128, 2, R], BF16, addr_space="Shared")

    GROUPS_KV = [[0, 1, 2, 3], [4, 5, 6, 7]]
    GROUPS_ALL = [CORE_IDS]

    with tile.TileContext(nc) as tc:
        with (
            tc.tile_pool(name="const", bufs=1) as cpool,
            tc.tile_pool(name="persist", bufs=1) as pp,
            tc.tile_pool(name="wts", bufs=2) as wp,
            tc.tile_pool(name="acts", bufs=1) as ap,
        ):
            ident = cpool.tile([128, 128], BF16)
            make_identity(nc, ident[:])
            epsb = cpool.tile([128, 1], F32)
            nc.vector.memset(epsb[:], EPS)

            h = pp.tile([128, RT, D], F32)
            mask_sb = pp.tile([128, NKT, 2, R], BF16)
            nc.sync.dma_start(mask_sb[:], mask2[:, :, :, :])

            def rms_scales(src, tag):
                """Returns (r, s) tiles [128, RT] f32: r = 1/sqrt(ms+eps),
                s = sqrt(ms+eps)."""
                ss = ap.tile([128, RT], F32, name=f"ss_{tag}", tag=f"ss_{tag}")
                for rt in range(RT):
                    sq = ap.tile([128, D], F32, name=f"sq_{tag}{rt}",
                                 tag="sq_scratch", bufs=2)
                    nc.scalar.activation(sq[:], src[:, rt, :], AF.Square,
                                         accum_out=ss[:, rt : rt + 1])
                sg = ap.tile([128, RT], F32, name=f"sg_{tag}", tag=f"sg_{tag}")
                nc.scalar.activation(sg[:], ss[:], AF.Sqrt, scale=1.0 / D,
                                     bias=epsb[:])
                rr = ap.tile([128, RT], F32, name=f"rr_{tag}", tag=f"rr_{tag}")
                nc.vector.reciprocal(rr[:], sg[:])
                return rr, sg

            def transpose_to(dst_ap, src_ap, psum_pool, tag):
                """PE-transpose one [128,128] bf16 tile src_ap -> dst_ap."""
                tp = psum_pool.tile([128, 128], BF16, name=f"tp_{tag}", tag="tp",
                                    bufs=2)
                nc.tensor.transpose(tp[:], src_ap, ident[:])
                nc.vector.tensor_copy(dst_ap, tp[:])

            # ================= embedding =================
            exT = ap.tile([128, 3, R], BF16)
            wem = ap.tile([128, 3, D], BF16)
            for j in range(3):
                nc.sync.dma_start(exT[:, j, :], ex_augT[j * 128:(j + 1) * 128, :])
                nc.sync.dma_start(wem[:, j, :], w_emb[j * 128:(j + 1) * 128, :])
            with tc.tile_pool(name="ps_emb", bufs=4, space="PSUM") as pse:
                for rt in range(RT):
                    for nch in range(2):
                        psum = pse.tile([128, 512], F32, tag="mm")
                        for j in range(3):
                            nc.tensor.matmul(
                                psum[:],
                                exT[:, j, rt * 128:(rt + 1) * 128],
                                wem[:, j, nch * 512:(nch + 1) * 512],
                                start=(j == 0), stop=(j == 2))
                        nc.scalar.copy(h[:, rt, nch * 512:(nch + 1) * 512], psum[:])
            if dbg:
                nc.sync.dma_start(dbg_t["dbg_h0"][:], h[:])

            # ================= layers =================
            for l in range(L):
                # ---- norm1 + double-rms for q/k ----
                n1 = wp.tile([128, D], F32, tag="n1")
                nc.sync.dma_start(n1[:], n1rep[l])
                r1, _ = rms_scales(h, f"n1_{l}")
                hnb = ap.tile([128, RT, D], BF16, tag="hnb")
                for rt in range(RT):
                    tmp = ap.tile([128, D], F32, name=f"tmp{l}{rt}",
                                  tag="tmp_f32", bufs=2)
                    nc.vector.tensor_scalar(tmp[:], h[:, rt, :],
                                            r1[:, rt : rt + 1], None,
                                            mybir.AluOpType.mult)
                    nc.vector.tensor_tensor(hnb[:, rt, :], tmp[:], n1[:],
                                            mybir.AluOpType.mult)
                r2, s2 = rms_scales(hnb, f"n2_{l}")
                qkb = ap.tile([128, RT, D], BF16, tag="qkb")
                for rt in range(RT):
                    nc.vector.tensor_scalar(qkb[:, rt, :], hnb[:, rt, :],
                                            r2[:, rt : rt + 1], None,
                                            mybir.AluOpType.mult)
                qkT = ap.tile([128, DKT, R], BF16, tag="xT")
                with tc.tile_pool(name=f"ps_tp{l}", bufs=2, space="PSUM") as ptp:
                    for rt in range(RT):
                        for kt in range(DKT):
                            transpose_to(qkT[:, kt, rt * 128:(rt + 1) * 128],
                                         qkb[:, rt, kt * 128:(kt + 1) * 128],
                                         ptp, f"qk{l}")

                # ---- Q, K, V projections (region-outer: one accumulation
                # group at a time; regions never interleave within a bank) ----
                qT = ap.tile([128, DKT, R], BF16, tag="qT")
                k_stage = ap.tile([128, 2, R], BF16, tag="k_stage")
                v_stage = ap.tile([128, RT, KV, HD], BF16, tag="v_stage")
                wq_sb = ap.tile([128, DKT, D], BF16, name=f"wq{l}", tag="wq_sb")
                wk_sb = ap.tile([128, DKT, KV * HD], BF16, name=f"wk{l}", tag="wk_sb")
                wv_sb = ap.tile([128, DKT, KV * HD], BF16, name=f"wv{l}", tag="wv_sb")
                for kt in range(DKT):
                    nc.sync.dma_start(wq_sb[:, kt, :],
                                      wqT[l, kt * 128:(kt + 1) * 128, :])
                    nc.sync.dma_start(wk_sb[:, kt, :],
                                      wkT[l, kt * 128:(kt + 1) * 128, :])
                    nc.sync.dma_start(wv_sb[:, kt, :],
                                      wvT[l, kt * 128:(kt + 1) * 128, :])
                with tc.tile_pool(name=f"ps_qkv{l}", bufs=4, space="PSUM") as pq:
                    for mt in range(DKT):  # q^T [qdim, rows]
                        psq = pq.tile([128, R], F32, tag="mm")
                        for kt in range(DKT):
                            nc.tensor.matmul(psq[:],
                                             wq_sb[:, kt, mt * 128:(mt + 1) * 128],
                                             qkT[:, kt, :],
                                             start=(kt == 0), stop=(kt == DKT - 1))
                        nc.scalar.copy(qT[:, mt, :], psq[:])
                    for mt in range(2):    # k^T [kvdim, rows]
                        psk = pq.tile([128, R], F32, tag="mm")
                        for kt in range(DKT):
                            nc.tensor.matmul(psk[:],
                                             wk_sb[:, kt, mt * 128:(mt + 1) * 128],
                                             qkT[:, kt, :],
                                             start=(kt == 0), stop=(kt == DKT - 1))
                        nc.scalar.copy(k_stage[:, mt, :], psk[:])
                    for rt in range(RT):   # v [rows, kvdim] (undo r2 via s2)
                        psv = pq.tile([128, KV * HD], F32, tag="mm")
                        for kt in range(DKT):
                            nc.tensor.matmul(psv[:],
                                             qkT[:, kt, rt * 128:(rt + 1) * 128],
                                             wv_sb[:, kt, :],
                                             start=(kt == 0), stop=(kt == DKT - 1))
                        nc.vector.tensor_scalar(
                            v_stage[:, rt, :, :],
                            psv[:].rearrange("p (a b) -> p a b", a=KV),
                            s2[:, rt : rt + 1], None, mybir.AluOpType.mult)

                # ---- KV all-gather (within batch group of 4 cores) ----
                for g in range(KV):
                    nc.sync.dma_start(k_in[l][:, g, :],
                                      k_stage[64 * (g % 2):64 * (g % 2) + 64, g // 2, :])
                nc.sync.dma_start(v_in[l][:], v_stage[:])
                nc.gpsimd.collective_compute(
                    "AllGather", mybir.AluOpType.bypass, replica_groups=GROUPS_KV,
                    ins=[k_in[l][:].opt()], outs=[k_g[l][:].opt()])
                nc.gpsimd.collective_compute(
                    "AllGather", mybir.AluOpType.bypass, replica_groups=GROUPS_KV,
                    ins=[v_in[l][:].opt()], outs=[v_g[l][:].opt()])

                kT_both = ap.tile([128, KV, T], BF16, tag="kT_both")
                v65 = ap.tile([128, NKT, KV, HD + 1], BF16, tag="v65")
                nc.vector.memset(v65[:], 1.0)
                for b in range(4):
                    for g in range(KV):
                        nc.sync.dma_start(kT_both[0:64, g, b * R:(b + 1) * R],
                                          k_g[l][b, :, g, :])
                        nc.sync.dma_start(kT_both[64:128, g, b * R:(b + 1) * R],
                                          k_g[l][b, :, g, :])
                    nc.sync.dma_start(v65[:, 2 * b:2 * b + 2, :, 0:HD], v_g[l][b])

                if dbg and l == 0:
                    nc.sync.dma_start(dbg_t["dbg_qT"][:], qT[:])
                    nc.sync.dma_start(dbg_t["dbg_kT"][:], kT_both[:])
                    nc.sync.dma_start(dbg_t["dbg_v65"][:], v65[:])

                # ---- attention ----
                oT = ap.tile([128, 8, R], BF16, tag="oT")
                with (
                    tc.tile_pool(name=f"ps_sc{l}", bufs=2, space="PSUM") as psc,
                    tc.tile_pool(name=f"ps_ot{l}", bufs=1, space="PSUM") as pso,
                ):
                    for g in range(KV):
                        po = [pso.tile([HD + 1, R], F32, name=f"po{g}{s}",
                                       tag=f"ot{s}") for s in range(4)]
                        for kt in range(NKT):
                            for j in range(2):
                                sc = psc.tile([128, 2, R], F32, tag="sc")
                                nc.tensor.matmul(
                                    sc[:],
                                    kT_both[64 * j:64 * j + 64, g,
                                            kt * 128:(kt + 1) * 128],
                                    qT[64 * j:64 * j + 64, 2 * g:2 * g + 2, :],
                                    start=True, stop=True)
                                er = ap.tile([128, 2, R], BF16, tag="expraw", bufs=4)
                                nc.scalar.activation(er[:], sc[:], AF.Exp,
                                                     scale=float(1.0 / np.sqrt(HD)))
                                # expM slot i holds q-head 4g + j + 2i
                                expM = ap.tile([128, 2, R], BF16, tag="expM", bufs=4)
                                nc.vector.tensor_tensor(
                                    expM[:], er[:], mask_sb[:, kt, :, :],
                                    mybir.AluOpType.mult)
                                for i in range(2):
                                    s = j + 2 * i
                                    nc.tensor.matmul(
                                        po[s][:], v65[:, kt, g, :], expM[:, i, :],
                                        start=(kt == 0), stop=(kt == NKT - 1))
                        for s in range(4):
                            hq = 4 * g + s
                            rec = ap.tile([1, R], F32, tag="rec", bufs=2)
                            nc.vector.reciprocal(rec[:], po[s][HD:HD + 1, :])
                            bcs = ap.tile([64, R], F32, tag="bcs", bufs=2)
                            nc.gpsimd.partition_broadcast(bcs[:], rec[:])
                            if hq % 2 == 0:
                                nc.vector.tensor_tensor(
                                    oT[0:64, hq // 2, :], po[s][0:HD, :], bcs[:],
                                    mybir.AluOpType.mult)
                            else:
                                otmp = ap.tile([64, R], BF16, tag="otmp", bufs=2)
                                nc.vector.tensor_tensor(otmp[:], po[s][0:HD, :],
                                                        bcs[:], mybir.AluOpType.mult)
                                nc.sync.dma_start(oT[64:128, hq // 2, :], otmp[:])

                if dbg and l == 0:
                    nc.sync.dma_start(dbg_t["dbg_oT"][:], oT[:])

                # ---- Wo + residual ----
                with tc.tile_pool(name=f"ps_wo{l}", bufs=1, space="PSUM") as pwo:
                    pswo = pwo.tile([128, RT, D], F32, tag="pswo")  # 4 banks
                    for kt in range(DKT):
                        wo_sl = ap.tile([128, D], BF16, name=f"wo{l}{kt}",
                                        tag="wsl", bufs=3)
                        nc.sync.dma_start(wo_sl[:], woT[l, kt * 128:(kt + 1) * 128, :])
                        for rt in range(RT):
                            for nch in range(2):
                                nc.tensor.matmul(
                                    pswo[:, rt, nch * 512:(nch + 1) * 512],
                                    oT[:, kt, rt * 128:(rt + 1) * 128],
                                    wo_sl[:, nch * 512:(nch + 1) * 512],
                                    start=(kt == 0), stop=(kt == DKT - 1))
                    for rt in range(RT):
                        nc.vector.tensor_tensor(h[:, rt, :], h[:, rt, :],
                                                pswo[:, rt, :], mybir.AluOpType.add)

                # ---- MLP (two DFF halves of 2048) ----
                r3, _ = rms_scales(h, f"n3_{l}")
                fnb = ap.tile([128, RT, D], BF16, tag="qkb")
                for rt in range(RT):
                    nc.vector.tensor_scalar(fnb[:, rt, :], h[:, rt, :],
                                            r3[:, rt : rt + 1], None,
                                            mybir.AluOpType.mult)
                fnT = ap.tile([128, DKT, R], BF16, tag="xT")
                with tc.tile_pool(name=f"ps_tpf{l}", bufs=2, space="PSUM") as ptf:
                    for rt in range(RT):
                        for kt in range(DKT):
                            transpose_to(fnT[:, kt, rt * 128:(rt + 1) * 128],
                                         fnb[:, rt, kt * 128:(kt + 1) * 128],
                                         ptf, f"fn{l}")

                for dh in range(2):
                    gs = ap.tile([128, RT, 2048], BF16, name=f"gs{l}{dh}", tag="gs")
                    us = ap.tile([128, RT, 2048], BF16, name=f"us{l}{dh}", tag="us")
                    for which, wsrc, dst in ((0, wgT, gs), (1, wuT, us)):
                        with tc.tile_pool(name=f"ps_ff{l}{dh}{which}", bufs=2,
                                          space="PSUM") as pff:
                            for grp in range(2):
                                psff = pff.tile([128, RT, 1024], F32, tag="psff")
                                col0 = dh * 2048 + grp * 1024
                                for kt in range(DKT):
                                    wsl = ap.tile([128, 1024], BF16,
                                                  name=f"w{which}{l}{dh}{grp}{kt}",
                                                  tag="wsl", bufs=3)
                                    nc.sync.dma_start(
                                        wsl[:], wsrc[l, kt * 128:(kt + 1) * 128,
                                                     col0:col0 + 1024])
                                    for rt in range(RT):
                                        for nch in range(2):
                                            nc.tensor.matmul(
                                                psff[:, rt,
                                                     nch * 512:(nch + 1) * 512],
                                                fnT[:, kt, rt * 128:(rt + 1) * 128],
                                                wsl[:, nch * 512:(nch + 1) * 512],
                                                start=(kt == 0),
                                                stop=(kt == DKT - 1))
                                for rt in range(RT):
                                    dap = dst[:, rt, grp * 1024:(grp + 1) * 1024]
                                    pap = psff[:, rt, :]
                                    if which == 0:
                                        nc.scalar.activation(dap, pap, AF.Silu)
                                    else:
                                        nc.vector.tensor_copy(dap, pap)
                    for rt in range(RT):
                        nc.vector.tensor_tensor(gs[:, rt, :], gs[:, rt, :],
                                                us[:, rt, :], mybir.AluOpType.mult)
                    mT = ap.tile([128, 16, R], BF16, name=f"mT{l}{dh}", tag="mT")
                    with tc.tile_pool(name=f"ps_tpm{l}{dh}", bufs=2,
                                      space="PSUM") as ptm:
                        for rt in range(RT):
                            for kt in range(16):
                                transpose_to(mT[:, kt, rt * 128:(rt + 1) * 128],
                                             gs[:, rt, kt * 128:(kt + 1) * 128],
                                             ptm, f"m{l}{dh}")
                    with tc.tile_pool(name=f"ps_wd{l}{dh}", bufs=1,
                                      space="PSUM") as pwd:
                        pswd = pwd.tile([128, RT, D], F32, tag="pswd")
                        for kt in range(16):
                            wdsl = ap.tile([128, D], BF16, name=f"wd{l}{dh}{kt}",
                                           tag="wsl", bufs=3)
                            nc.sync.dma_start(
                                wdsl[:], wdT[l, (dh * 16 + kt) * 128:
                                             (dh * 16 + kt + 1) * 128, :])
                            for rt in range(RT):
                                for nch in range(2):
                                    nc.tensor.matmul(
                                        pswd[:, rt, nch * 512:(nch + 1) * 512],
                                        mT[:, kt, rt * 128:(rt + 1) * 128],
                                        wdsl[:, nch * 512:(nch + 1) * 512],
                                        start=(kt == 0), stop=(kt == 15))
                        for rt in range(RT):
                            nc.vector.tensor_tensor(h[:, rt, :], h[:, rt, :],
                                                    pswd[:, rt, :],
                                                    mybir.AluOpType.add)
                if dbg:
                    nc.sync.dma_start(dbg_t[f"dbg_h{l + 1}"][:], h[:])

            # ================= final norm + patch + logits =================
            r4, _ = rms_scales(h, "fin")
            hfb = ap.tile([128, RT, D], BF16, tag="hnb")
            for rt in range(RT):
                nc.vector.tensor_scalar(hfb[:, rt, :], h[:, rt, :],
                                        r4[:, rt : rt + 1], None,
                                        mybir.AluOpType.mult)
            hfT = ap.tile([128, DKT, R], BF16, tag="xT")
            with tc.tile_pool(name="ps_tph", bufs=2, space="PSUM") as pth:
                for rt in range(RT):
                    for kt in range(DKT):
                        transpose_to(hfT[:, kt, rt * 128:(rt + 1) * 128],
                                     hfb[:, rt, kt * 128:(kt + 1) * 128],
                                     pth, "hf")
            pt_sb = wp.tile([128, DKT, P], BF16, tag="pt_sb", bufs=1)
            for kt in range(DKT):
                nc.sync.dma_start(pt_sb[:, kt, :], patchT[kt * 128:(kt + 1) * 128, :])
            hp_stage = ap.tile([128, 2, R], BF16, tag="hp_stage")
            with tc.tile_pool(name="ps_hp", bufs=2, space="PSUM") as php:
                for mt in range(2):
                    psp = php.tile([128, R], F32, tag="mm")
                    for kt in range(DKT):
                        nc.tensor.matmul(psp[:], pt_sb[:, kt, mt * 128:(mt + 1) * 128],
                                         hfT[:, kt, :],
                                         start=(kt == 0), stop=(kt == DKT - 1))
                    nc.scalar.copy(hp_stage[:, mt, :], psp[:])
            if dbg:
                nc.sync.dma_start(dbg_t["dbg_hp"][:], hp_stage[:])
            nc.sync.dma_start(hp_in[:], hp_stage[:])
            nc.gpsimd.collective_compute(
                "AllGather", mybir.AluOpType.bypass, replica_groups=GROUPS_ALL,
                ins=[hp_in[:].opt()], outs=[hp_g[:].opt()])
            hpT = ap.tile([128, 16, R], BF16, tag="mT")
            for rbk in range(8):
                nc.sync.dma_start(hpT[:, 2 * rbk:2 * rbk + 2, :], hp_g[rbk])
            ec0 = ap.tile([128, VSH], BF16, tag="gs")
            ec1 = ap.tile([128, VSH], BF16, tag="us")
            nc.sync.dma_start(ec0[:], ecT[0:128, :])
            nc.sync.dma_start(ec1[:], ecT[128:256, :])
            ec = [ec0, ec1]
            with tc.tile_pool(name="ps_lg", bufs=2, space="PSUM") as plg:
                for rbk in range(8):
                    for s in range(2):
                        for half in range(2):
                            plt = plg.tile([128, 4, 512], F32, tag="lg")
                            for kt in range(2):
                                for nq in range(4):
                                    nc.tensor.matmul(
                                        plt[:, nq, 0:500],
                                        hpT[:, 2 * rbk + kt, s * 128:(s + 1) * 128],
                                        ec[kt][:, half * 2000 + nq * 500:
                                               half * 2000 + (nq + 1) * 500],
                                        start=(kt == 0), stop=(kt == 1))
                            lg_sb = ap.tile([128, 2000], BF16, tag="lg_sb", bufs=3)
                            for nq in range(4):
                                if half == 0:
                                    nc.scalar.copy(lg_sb[:, nq * 500:(nq + 1) * 500],
                                                   plt[:, nq, 0:500])
                                else:
                                    nc.vector.tensor_copy(
                                        lg_sb[:, nq * 500:(nq + 1) * 500],
                                        plt[:, nq, 0:500])
                            nc.sync.dma_start(
                                out[rbk * 256 + s * 128: rbk * 256 + (s + 1) * 128,
                                    half * 2000:(half + 1) * 2000], lg_sb[:])

    nc.compile()
    _cache[key] = nc
    return nc


def _prep_inputs(x, z0, E, W_embed_up, W_z0, patch_W, final_norm_w,
                 norm1_w, q_norm_w, k_norm_w, norm2_w,
                 Wq, Wk, Wv, Wo, Wg, Wu, Wd):
    bf = ml_dtypes.bfloat16
    f32 = np.float32
    E = np.asarray(E, f32)
    x = np.asarray(x).astype(np.int64).reshape(B * T)

    zproj = np.asarray(z0, f32) @ np.asarray(W_z0, f32).T  # (B, D)

    def t(a):
        return np.ascontiguousarray(np.asarray(a, f32).T).astype(bf)

    wqTn = np.stack([t(np.asarray(Wq[l], f32) * np.asarray(q_norm_w[l], f32)[None, :])
                     for l in range(L)])
    wkTn = np.stack([t(np.asarray(Wk[l], f32) * np.asarray(k_norm_w[l], f32)[None, :])
                     for l in range(L)])
    wvTn = np.stack([t(Wv[l]) for l in range(L)])
    woTn = np.stack([t(Wo[l]) for l in range(L)])
    wgTn = np.stack([t(np.asarray(Wg[l], f32) * np.asarray(norm2_w[l], f32)[None, :])
                     for l in range(L)])
    wuTn = np.stack([t(np.asarray(Wu[l], f32) * np.asarray(norm2_w[l], f32)[None, :])
                     for l in range(L)])
    wdTn = np.stack([t(Wd[l]) for l in range(L)])
    patchTn = t(np.asarray(patch_W, f32) * np.asarray(final_norm_w, f32)[None, :])
    n1rep = np.stack([np.broadcast_to(np.asarray(norm1_w[l], f32), (128, D)).copy()
                      for l in range(L)])

    wembT = np.asarray(W_embed_up, f32).T  # (P, D)
    in_maps = []
    for c in range(N_CORES):
        rows = x[c * R:(c + 1) * R]
        ex = E[rows]                       # (R, P)
        ex_augT = np.zeros((384, R), f32)
        ex_augT[:P, :] = ex.T
        ex_augT[P, :] = 1.0
        w_emb = np.zeros((384, D), f32)
        w_emb[:P, :] = wembT
        w_emb[P, :] = zproj[c // 4]

        qoff = (c % 4) * R
        kidx = np.arange(NKT * 128)
        qidx = qoff + np.arange(R)
        m = (kidx[:, None] <= qidx[None, :]).astype(f32)   # (1024, R)
        mask2 = np.broadcast_to(
            m.reshape(NKT, 128, 1, R), (NKT, 128, 2, R)).astype(bf).copy()

        ecTn = np.ascontiguousarray(E[c * VSH:(c + 1) * VSH].T).astype(bf)

        in_maps.append({
            "ex_augT": ex_augT.astype(bf), "w_emb": w_emb.astype(bf),
            "wqT": wqTn, "wkT": wkTn, "wvT": wvTn, "woT": woTn,
            "wgT": wgTn, "wuT": wuTn, "wdT": wdTn,
            "n1rep": n1rep, "mask2": mask2, "patchT": patchTn, "ecT": ecTn,
        })
    return in_maps


last_exec_ns = None


def kernel(**inputs) -> np.ndarray:
    global last_exec_ns
    in_maps = _prep_inputs(**inputs)
    nc = build()
    trace = bool(int(os.environ.get("TRN_PROFILE", "0")))
    kw = {}
    if trace:
        try:
            import prof_shim
            prof_shim.install()
            kw = dict(trace=True, tmpdir=os.environ.get("TRN_TRACE_DIR", None))
        except Exception:
            kw = {}
    res = run_bass_kernel_spmd(nc, in_maps, CORE_IDS, **kw)
    last_exec_ns = res.exec_time_ns
    parts = [np.asarray(res.results[c]["logits"]).astype(np.float32)
             for c in range(N_CORES)]
    return np.concatenate(parts, axis=1).reshape(B, T, V)


# revision 28
# speedup vs baseline: 1.1246x; 1.1246x over previous
"""Trainium2 distributed kernel for nn_ARDecoder (2x1024 tokens, D=1024,
H=16/KV=4 GQA, DFF=4096, V=32000, P=256, 4 layers).

Strategy: data-parallel over the 2048 (batch*seq) rows -- 256 rows per core.
Weights are replicated (bf16), activations stay SBUF-resident. Causal
attention uses a per-batch AllGather of K^T and V (replica groups
[[0..3],[4..7]]). The logits GEMM is vocab-sharded: h@patch_W.T is
all-gathered (tiny) and each core computes its 4000-column slice of E.
Host does the embedding gather E[x], norm-weight folding, transposes to
K-major weight layouts, and the final concat over vocab shards.
"""

import os
import numpy as np
import ml_dtypes

import concourse.bass as bass
import concourse.bacc as bacc
import concourse.mybir as mybir
import concourse.tile as tile
from concourse.bass_utils import run_bass_kernel_spmd
from concourse.masks import make_identity

BF16 = mybir.dt.bfloat16
F32 = mybir.dt.float32
AF = mybir.ActivationFunctionType

N_CORES = 8
CORE_IDS = list(range(N_CORES))
B, T, D, H, KV, HD, DFF, V, P, DLAT, L = 2, 1024, 1024, 16, 4, 64, 4096, 32000, 256, 512, 4
EPS = 1e-6
R = 256            # rows per core
RT = 2             # row tiles of 128
DKT = D // 128     # 8 k-tiles over D
VSH = V // N_CORES # 4000 vocab columns per core
NKT = 8            # key tiles of 128 within a batch

_cache = {}


def build(dbg=False):
    key = ("nc", dbg)
    if key in _cache:
        return _cache[key]
    nc = bacc.Bacc("TRN2", target_bir_lowering=False, debug=False,
                   num_devices=N_CORES)
    dbg_t = {}
    if dbg:
        for name, shape, dt in [
            ("dbg_h0", [128, RT, D], F32), ("dbg_h1", [128, RT, D], F32),
            ("dbg_h2", [128, RT, D], F32), ("dbg_h3", [128, RT, D], F32),
            ("dbg_h4", [128, RT, D], F32),
            ("dbg_qT", [128, DKT, R], BF16), ("dbg_kT", [128, KV, T], BF16),
            ("dbg_v65", [128, NKT, KV, HD + 1], BF16),
            ("dbg_oT", [128, 8, R], BF16), ("dbg_hp", [128, 2, R], BF16),
        ]:
            dbg_t[name] = nc.dram_tensor(name, shape, dt, kind="ExternalOutput")

    # ---- parameters (per-core inputs) ----
    ex_augT = nc.dram_tensor("ex_augT", [384, R], BF16, kind="ExternalInput")
    w_emb = nc.dram_tensor("w_emb", [384, D], BF16, kind="ExternalInput")
    wqT = nc.dram_tensor("wqT", [L, D, D], BF16, kind="ExternalInput")
    wkT = nc.dram_tensor("wkT", [L, D, KV * HD], BF16, kind="ExternalInput")
    wvT = nc.dram_tensor("wvT", [L, D, KV * HD], BF16, kind="ExternalInput")
    woT = nc.dram_tensor("woT", [L, D, D], BF16, kind="ExternalInput")
    wgT = nc.dram_tensor("wgT", [L, D, DFF], BF16, kind="ExternalInput")
    wuT = nc.dram_tensor("wuT", [L, D, DFF], BF16, kind="ExternalInput")
    wdT = nc.dram_tensor("wdT", [L, DFF, D], BF16, kind="ExternalInput")
    n1rep = nc.dram_tensor("n1rep", [L, 128, D], F32, kind="ExternalInput")
    mask2 = nc.dram_tensor("mask2", [NKT, 128, 2, R], BF16, kind="ExternalInput")
    patchT = nc.dram_tensor("patchT", [D, P], BF16, kind="ExternalInput")
    ecT = nc.dram_tensor("ecT", [P, VSH], BF16, kind="ExternalInput")
    out = nc.dram_tensor("logits", [B * T, VSH], BF16, kind="ExternalOutput")

    # ---- internal DRAM (collective bounce buffers) ----
    # kv bounce layout: [0:65536] = K^T [64, KV, R]; [65536:] = V [128, RT, KV, HD]
    kv_in, kv_g = [], []
    for l in range(L):
        kv_in.append(nc.dram_tensor(f"kv_in{l}", [2 * 65536], BF16))
        kv_g.append(nc.dram_tensor(f"kv_g{l}", [4, 2 * 65536], BF16))
    hp_in = nc.dram_tensor("hp_in", [128, 2, R], BF16)
    hp_g = nc.dram_tensor("hp_g", [8, 128, 2, R], BF16, addr_space="Shared")

    GROUPS_KV = [[0, 1, 2, 3], [4, 5, 6, 7]]
    GROUPS_ALL = [CORE_IDS]

    with tile.TileContext(nc) as tc:
        with (
            tc.tile_pool(name="const", bufs=1) as cpool,
            tc.tile_pool(name="persist", bufs=1) as pp,
            tc.tile_pool(name="wts", bufs=2) as wp,
            tc.tile_pool(name="acts", bufs=1) as ap,
        ):
            ident = cpool.tile([128, 128], BF16)
            make_identity(nc, ident[:])
            epsb = cpool.tile([128, 1], F32)
            nc.vector.memset(epsb[:], EPS)

            h = pp.tile([128, RT, D], F32)
            mask_sb = pp.tile([128, NKT, 2, R], BF16)
            nc.sync.dma_start(mask_sb[:], mask2[:, :, :, :])

            def rms_scales(src, tag):
                """Returns (r, s) tiles [128, RT] f32: r = 1/sqrt(ms+eps),
                s = sqrt(ms+eps)."""
                ss = ap.tile([128, RT], F32, name=f"ss_{tag}", tag=f"ss_{tag}")
                for rt in range(RT):
                    sq = ap.tile([128, D], F32, name=f"sq_{tag}{rt}",
                                 tag="sq_scratch", bufs=2)
                    nc.scalar.activation(sq[:], src[:, rt, :], AF.Square,
                                         accum_out=ss[:, rt : rt + 1])
                sg = ap.tile([128, RT], F32, name=f"sg_{tag}", tag=f"sg_{tag}")
                rr = ap.tile([128, RT], F32, name=f"rr_{tag}", tag=f"rr_{tag}")
                for rt in range(RT):  # per-rt so downstream rt0 work can start
                    nc.scalar.activation(sg[:, rt : rt + 1], ss[:, rt : rt + 1],
                                         AF.Sqrt, scale=1.0 / D, bias=epsb[:])
                    nc.vector.reciprocal(rr[:, rt : rt + 1], sg[:, rt : rt + 1])
                return rr, sg

            def transpose_to(dst_ap, src_ap, psum_pool, tag):
                """PE-transpose one [128,128] bf16 tile src_ap -> dst_ap."""
                tp = psum_pool.tile([128, 128], BF16, name=f"tp_{tag}", tag="tp",
                                    bufs=2)
                nc.tensor.transpose(tp[:], src_ap, ident[:])
                nc.vector.tensor_copy(dst_ap, tp[:])

            # ================= embedding =================
            exT = ap.tile([128, 3, R], BF16)
            wem = ap.tile([128, 3, D], BF16)
            nc.sync.dma_start(exT[:], ex_augT[:].rearrange("(j p) d -> p j d", p=128))
            nc.sync.dma_start(wem[:], w_emb[:].rearrange("(j p) d -> p j d", p=128))
            with tc.tile_pool(name="ps_emb", bufs=4, space="PSUM") as pse:
                for rt in range(RT):
                    for nch in range(2):
                        psum = pse.tile([128, 512], F32, tag="mm")
                        for j in range(3):
                            nc.tensor.matmul(
                                psum[:],
                                exT[:, j, rt * 128:(rt + 1) * 128],
                                wem[:, j, nch * 512:(nch + 1) * 512],
                                start=(j == 0), stop=(j == 2))
                        nc.scalar.copy(h[:, rt, nch * 512:(nch + 1) * 512], psum[:])
            if dbg:
                nc.sync.dma_start(dbg_t["dbg_h0"][:], h[:])

            # ================= layers =================
            for l in range(L):
                # ---- norm1 + double-rms for q/k ----
                n1 = wp.tile([128, D], F32, tag="n1")
                nc.sync.dma_start(n1[:], n1rep[l])
                r1, _ = rms_scales(h, f"n1_{l}")
                hnb = ap.tile([128, RT, D], BF16, tag="hnb")
                for rt in range(RT):
                    tmp = ap.tile([128, D], F32, name=f"tmp{l}{rt}",
                                  tag="tmp_f32", bufs=2)
                    nc.vector.tensor_scalar(tmp[:], h[:, rt, :],
                                            r1[:, rt : rt + 1], None,
                                            mybir.AluOpType.mult)
                    nc.vector.tensor_tensor(hnb[:, rt, :], tmp[:], n1[:],
                                            mybir.AluOpType.mult)
                r2, s2 = rms_scales(hnb, f"n2_{l}")
                qkb = ap.tile([128, RT, D], BF16, tag="qkb")
                for rt in range(RT):
                    nc.vector.tensor_scalar(qkb[:, rt, :], hnb[:, rt, :],
                                            r2[:, rt : rt + 1], None,
                                            mybir.AluOpType.mult)
                qkT = ap.tile([128, DKT, R], BF16, tag="xT")
                with tc.tile_pool(name=f"ps_tp{l}", bufs=2, space="PSUM") as ptp:
                    for rt in range(RT):
                        for kt in range(DKT):
                            transpose_to(qkT[:, kt, rt * 128:(rt + 1) * 128],
                                         qkb[:, rt, kt * 128:(kt + 1) * 128],
                                         ptp, f"qk{l}")

                # ---- K, V first (kick the all-gather early), then Q ----
                qT = ap.tile([128, DKT, R], BF16, tag="qT")
                k_stage = ap.tile([128, 2, R], BF16, tag="k_stage")
                v_stage = ap.tile([128, RT, KV, HD], BF16, tag="v_stage")
                wk_sb = ap.tile([128, DKT, KV * HD], BF16, name=f"wk{l}", tag="wk_sb")
                wv_sb = ap.tile([128, DKT, KV * HD], BF16, name=f"wv{l}", tag="wv_sb")
                nc.sync.dma_start(wk_sb[:],
                                  wkT[l].rearrange("(kt p) d -> p kt d", p=128))
                nc.sync.dma_start(wv_sb[:],
                                  wvT[l].rearrange("(kt p) d -> p kt d", p=128))
                k_ap = kv_in[l][0:65536].rearrange("(h g r) -> h g r", h=64, g=KV)
                v_ap = kv_in[l][65536:].rearrange("(p t g x) -> p t g x",
                                                  p=128, t=RT, g=KV)
                with tc.tile_pool(name=f"ps_qkv{l}", bufs=4, space="PSUM") as pq:
                    for mt in range(2):    # k^T [kvdim, rows]
                        psk = pq.tile([128, R], F32, tag="mm")
                        for kt in range(DKT):
                            nc.tensor.matmul(psk[:],
                                             wk_sb[:, kt, mt * 128:(mt + 1) * 128],
                                             qkT[:, kt, :],
                                             start=(kt == 0), stop=(kt == DKT - 1))
                        nc.scalar.copy(k_stage[:, mt, :], psk[:])
                    for rt in range(RT):   # v [rows, kvdim] (undo r2 via s2)
                        psv = pq.tile([128, KV * HD], F32, tag="mm")
                        for kt in range(DKT):
                            nc.tensor.matmul(psv[:],
                                             qkT[:, kt, rt * 128:(rt + 1) * 128],
                                             wv_sb[:, kt, :],
                                             start=(kt == 0), stop=(kt == DKT - 1))
                        nc.vector.tensor_scalar(
                            v_stage[:, rt, :, :],
                            psv[:].rearrange("p (a b) -> p a b", a=KV),
                            s2[:, rt : rt + 1], None, mybir.AluOpType.mult)
                    # stage K/V to DRAM and kick the gather
                    for g in range(KV):
                        nc.sync.dma_start(
                            k_ap[:, g, :],
                            k_stage[64 * (g % 2):64 * (g % 2) + 64, g // 2, :])
                    nc.sync.dma_start(v_ap, v_stage[:])
                    nc.gpsimd.collective_compute(
                        "AllGather", mybir.AluOpType.bypass,
                        replica_groups=GROUPS_KV,
                        ins=[kv_in[l][:].opt()], outs=[kv_g[l][:].opt()])
                    # Q while the gather is in flight
                    wqb = ap.tile([128, DKT, D], BF16, name=f"wq{l}", tag="wbig",
                                  bufs=2)
                    nc.sync.dma_start(wqb[:],
                                      wqT[l].rearrange("(kt p) d -> p kt d", p=128))
                    for mt in range(DKT):  # q^T [qdim, rows]
                        psq = pq.tile([128, R], F32, tag="mm")
                        for kt in range(DKT):
                            nc.tensor.matmul(psq[:],
                                             wqb[:, kt, mt * 128:(mt + 1) * 128],
                                             qkT[:, kt, :],
                                             start=(kt == 0), stop=(kt == DKT - 1))
                        nc.scalar.copy(qT[:, mt, :], psq[:])

                kT_both = ap.tile([128, KV, T], BF16, tag="kT_both")
                v65 = ap.tile([128, NKT, KV, HD + 1], BF16, tag="v65")
                nc.vector.memset(v65[:], 1.0)
                for b in range(4):
                    kg_ap = kv_g[l][b, 0:65536].rearrange("(h g r) -> h g r",
                                                          h=64, g=KV)
                    vg_ap = kv_g[l][b, 65536:].rearrange("(p t g x) -> p t g x",
                                                         p=128, t=RT, g=KV)
                    nc.sync.dma_start(kT_both[0:64, :, b * R:(b + 1) * R], kg_ap)
                    nc.sync.dma_start(kT_both[64:128, :, b * R:(b + 1) * R], kg_ap)
                    nc.sync.dma_start(v65[:, 2 * b:2 * b + 2, :, 0:HD], vg_ap)

                if dbg and l == 0:
                    nc.sync.dma_start(dbg_t["dbg_qT"][:], qT[:])
                    nc.sync.dma_start(dbg_t["dbg_kT"][:], kT_both[:])
                    nc.sync.dma_start(dbg_t["dbg_v65"][:], v65[:])

                # ---- attention ----
                oT = ap.tile([128, 8, R], BF16, tag="oT")
                with (
                    tc.tile_pool(name=f"ps_sc{l}", bufs=2, space="PSUM") as psc,
                    tc.tile_pool(name=f"ps_ot{l}", bufs=1, space="PSUM") as pso,
                ):
                    for g in range(KV):
                        po = [pso.tile([HD + 1, R], F32, name=f"po{g}{s}",
                                       tag=f"ot{s}") for s in range(4)]
                        for kt in range(NKT):
                            # scores^T for all 4 q-heads of group g: region j
                            # is one psum bank; expM slot (j, i) = head 4g+j+2i
                            sc = psc.tile([128, 2, 2, R], F32, tag="sc")
                            for j in range(2):
                                nc.tensor.matmul(
                                    sc[:, j, :, :],
                                    kT_both[64 * j:64 * j + 64, g,
                                            kt * 128:(kt + 1) * 128],
                                    qT[64 * j:64 * j + 64, 2 * g:2 * g + 2, :],
                                    start=True, stop=True)
                            er = ap.tile([128, 2, 2, R], BF16, tag="expraw", bufs=4)
                            nc.scalar.activation(er[:], sc[:], AF.Exp,
                                                 scale=float(1.0 / np.sqrt(HD)))
                            expM = ap.tile([128, 2, 2, R], BF16, tag="expM", bufs=4)
                            nc.vector.tensor_tensor(
                                expM[:], er[:],
                                mask_sb[:, kt, 0:1, :].unsqueeze(2)
                                .broadcast_to((128, 2, 2, R)),
                                mybir.AluOpType.mult)
                            for s in range(4):
                                nc.tensor.matmul(
                                    po[s][:], v65[:, kt, g, :],
                                    expM[:, s % 2, s // 2, :],
                                    start=(kt == 0), stop=(kt == NKT - 1))
                        for s in range(4):
                            hq = 4 * g + s
                            den = ap.tile([1, R], F32, tag="den", bufs=2)
                            nc.scalar.copy(den[:], po[s][HD:HD + 1, :])
                            bcs = ap.tile([64, R], F32, tag="bcs", bufs=2)
                            nc.gpsimd.partition_broadcast(bcs[:], den[:])
                            rec = ap.tile([64, R], F32, tag="rec", bufs=2)
                            nc.vector.reciprocal(rec[:], bcs[:])
                            if hq % 2 == 0:
                                nc.vector.tensor_tensor(
                                    oT[0:64, hq // 2, :], po[s][0:HD, :], rec[:],
                                    mybir.AluOpType.mult)
                            else:
                                otmp = ap.tile([64, R], BF16, tag="otmp", bufs=2)
                                nc.vector.tensor_tensor(otmp[:], po[s][0:HD, :],
                                                        rec[:], mybir.AluOpType.mult)
                                nc.sync.dma_start(oT[64:128, hq // 2, :], otmp[:])

                if dbg and l == 0:
                    nc.sync.dma_start(dbg_t["dbg_oT"][:], oT[:])

                # ---- Wo + residual ----
                wob = ap.tile([128, DKT, D], BF16, name=f"wo{l}", tag="wbig",
                              bufs=2)
                nc.sync.dma_start(wob[:],
                                  woT[l].rearrange("(kt p) d -> p kt d", p=128))
                with tc.tile_pool(name=f"ps_wo{l}", bufs=1, space="PSUM") as pwo:
                    pswo = pwo.tile([128, RT, D], F32, tag="pswo")  # 4 banks
                    for kt in range(DKT):
                        for rt in range(RT):
                            for nch in range(2):
                                nc.tensor.matmul(
                                    pswo[:, rt, nch * 512:(nch + 1) * 512],
                                    oT[:, kt, rt * 128:(rt + 1) * 128],
                                    wob[:, kt, nch * 512:(nch + 1) * 512],
                                    start=(kt == 0), stop=(kt == DKT - 1))
                    for rt in range(RT):
                        nc.vector.tensor_tensor(h[:, rt, :], h[:, rt, :],
                                                pswo[:, rt, :], mybir.AluOpType.add)

                # ---- MLP (two DFF halves of 2048) ----
                r3, _ = rms_scales(h, f"n3_{l}")
                fnb = ap.tile([128, RT, D], BF16, tag="qkb")
                for rt in range(RT):
                    nc.vector.tensor_scalar(fnb[:, rt, :], h[:, rt, :],
                                            r3[:, rt : rt + 1], None,
                                            mybir.AluOpType.mult)
                fnT = ap.tile([128, DKT, R], BF16, tag="xT")
                with tc.tile_pool(name=f"ps_tpf{l}", bufs=2, space="PSUM") as ptf:
                    for rt in range(RT):
                        for kt in range(DKT):
                            transpose_to(fnT[:, kt, rt * 128:(rt + 1) * 128],
                                         fnb[:, rt, kt * 128:(kt + 1) * 128],
                                         ptf, f"fn{l}")

                for dh in range(2):
                    gs = ap.tile([128, RT, 2048], BF16, name=f"gs{l}{dh}", tag="gs")
                    us = ap.tile([128, RT, 2048], BF16, name=f"us{l}{dh}", tag="us")
                    for which, wsrc, dst in ((0, wgT, gs), (1, wuT, us)):
                        with tc.tile_pool(name=f"ps_ff{l}{dh}{which}", bufs=2,
                                          space="PSUM") as pff:
                            for grp in range(2):
                                psff = pff.tile([128, RT, 1024], F32, tag="psff")
                                col0 = dh * 2048 + grp * 1024
                                wfb = ap.tile([128, DKT, 1024], BF16,
                                              name=f"w{which}{l}{dh}{grp}",
                                              tag="wbig", bufs=2)
                                nc.sync.dma_start(
                                    wfb[:],
                                    wsrc[l].rearrange("(kt p) d -> p kt d",
                                                      p=128)[:, :, col0:col0 + 1024])
                                for kt in range(DKT):
                                    for rt in range(RT):
                                        for nch in range(2):
                                            nc.tensor.matmul(
                                                psff[:, rt,
                                                     nch * 512:(nch + 1) * 512],
                                                fnT[:, kt, rt * 128:(rt + 1) * 128],
                                                wfb[:, kt,
                                                    nch * 512:(nch + 1) * 512],
                                                start=(kt == 0),
                                                stop=(kt == DKT - 1))
                                for rt in range(RT):
                                    dap = dst[:, rt, grp * 1024:(grp + 1) * 1024]
                                    pap = psff[:, rt, :]
                                    if which == 0:
                                        nc.scalar.activation(dap, pap, AF.Silu)
                                    else:
                                        nc.vector.tensor_copy(dap, pap)
                    for rt in range(RT):
                        nc.vector.tensor_tensor(gs[:, rt, :], gs[:, rt, :],
                                                us[:, rt, :], mybir.AluOpType.mult)
                    mT = ap.tile([128, 16, R], BF16, name=f"mT{l}{dh}", tag="mT")
                    with tc.tile_pool(name=f"ps_tpm{l}{dh}", bufs=2,
                                      space="PSUM") as ptm:
                        for rt in range(RT):
                            for kt in range(16):
                                transpose_to(mT[:, kt, rt * 128:(rt + 1) * 128],
                                             gs[:, rt, kt * 128:(kt + 1) * 128],
                                             ptm, f"m{l}{dh}")
                    with tc.tile_pool(name=f"ps_wd{l}{dh}", bufs=1,
                                      space="PSUM") as pwd:
                        pswd = pwd.tile([128, RT, D], F32, tag="pswd")
                        wdr = wdT[l].rearrange("(kt p) d -> p kt d", p=128)
                        for blk in range(2):
                            wdb = ap.tile([128, DKT, D], BF16,
                                          name=f"wd{l}{dh}{blk}",
                                          tag="wbig", bufs=2)
                            nc.sync.dma_start(
                                wdb[:], wdr[:, dh * 16 + blk * 8:
                                            dh * 16 + (blk + 1) * 8, :])
                            for kt8 in range(DKT):
                                kt = blk * 8 + kt8
                                for rt in range(RT):
                                    for nch in range(2):
                                        nc.tensor.matmul(
                                            pswd[:, rt, nch * 512:(nch + 1) * 512],
                                            mT[:, kt, rt * 128:(rt + 1) * 128],
                                            wdb[:, kt8,
                                                nch * 512:(nch + 1) * 512],
                                            start=(kt == 0), stop=(kt == 15))
                        for rt in range(RT):
                            nc.vector.tensor_tensor(h[:, rt, :], h[:, rt, :],
                                                    pswd[:, rt, :],
                                                    mybir.AluOpType.add)
                if dbg:
                    nc.sync.dma_start(dbg_t[f"dbg_h{l + 1}"][:], h[:])

            # ================= final norm + patch + logits =================
            r4, _ = rms_scales(h, "fin")
            hfb = ap.tile([128, RT, D], BF16, tag="hnb")
            for rt in range(RT):
                nc.vector.tensor_scalar(hfb[:, rt, :], h[:, rt, :],
                                        r4[:, rt : rt + 1], None,
                                        mybir.AluOpType.mult)
            hfT = ap.tile([128, DKT, R], BF16, tag="xT")
            with tc.tile_pool(name="ps_tph", bufs=2, space="PSUM") as pth:
                for rt in range(RT):
                    for kt in range(DKT):
                        transpose_to(hfT[:, kt, rt * 128:(rt + 1) * 128],
                                     hfb[:, rt, kt * 128:(kt + 1) * 128],
                                     pth, "hf")
            pt_sb = wp.tile([128, DKT, P], BF16, tag="pt_sb", bufs=1)
            for kt in range(DKT):
                nc.sync.dma_start(pt_sb[:, kt, :], patchT[kt * 128:(kt + 1) * 128, :])
            hp_stage = ap.tile([128, 2, R], BF16, tag="hp_stage")
            with tc.tile_pool(name="ps_hp", bufs=2, space="PSUM") as php:
                for mt in range(2):
                    psp = php.tile([128, R], F32, tag="mm")
                    for kt in range(DKT):
                        nc.tensor.matmul(psp[:], pt_sb[:, kt, mt * 128:(mt + 1) * 128],
                                         hfT[:, kt, :],
                                         start=(kt == 0), stop=(kt == DKT - 1))
                    nc.scalar.copy(hp_stage[:, mt, :], psp[:])
            if dbg:
                nc.sync.dma_start(dbg_t["dbg_hp"][:], hp_stage[:])
            nc.sync.dma_start(hp_in[:], hp_stage[:])
            nc.gpsimd.collective_compute(
                "AllGather", mybir.AluOpType.bypass, replica_groups=GROUPS_ALL,
                ins=[hp_in[:].opt()], outs=[hp_g[:].opt()])
            hpT = ap.tile([128, 16, R], BF16, tag="mT")
            for rbk in range(8):
                nc.sync.dma_start(hpT[:, 2 * rbk:2 * rbk + 2, :], hp_g[rbk])
            ec0 = ap.tile([128, VSH], BF16, tag="gs")
            ec1 = ap.tile([128, VSH], BF16, tag="us")
            nc.sync.dma_start(ec0[:], ecT[0:128, :])
            nc.sync.dma_start(ec1[:], ecT[128:256, :])
            ec = [ec0, ec1]
            with tc.tile_pool(name="ps_lg", bufs=2, space="PSUM") as plg:
                for rbk in range(8):
                    for s in range(2):
                        for half in range(2):
                            plt = plg.tile([128, 4, 512], F32, tag="lg")
                            for kt in range(2):
                                for nq in range(4):
                                    nc.tensor.matmul(
                                        plt[:, nq, 0:500],
                                        hpT[:, 2 * rbk + kt, s * 128:(s + 1) * 128],
                                        ec[kt][:, half * 2000 + nq * 500:
                                               half * 2000 + (nq + 1) * 500],
                                        start=(kt == 0), stop=(kt == 1))
                            lg_sb = ap.tile([128, 2000], BF16, tag="lg_sb", bufs=3)
                            for nq in range(4):
                                if half == 0:
                                    nc.scalar.copy(lg_sb[:, nq * 500:(nq + 1) * 500],
                                                   plt[:, nq, 0:500])
                                else:
                                    nc.vector.tensor_copy(
                                        lg_sb[:, nq * 500:(nq + 1) * 500],
                                        plt[:, nq, 0:500])
                            nc.sync.dma_start(
                                out[rbk * 256 + s * 128: rbk * 256 + (s + 1) * 128,
                                    half * 2000:(half + 1) * 2000], lg_sb[:])

    nc.compile()
    _cache[key] = nc
    return nc


def _prep_inputs(x, z0, E, W_embed_up, W_z0, patch_W, final_norm_w,
                 norm1_w, q_norm_w, k_norm_w, norm2_w,
                 Wq, Wk, Wv, Wo, Wg, Wu, Wd):
    bf = ml_dtypes.bfloat16
    f32 = np.float32
    E = np.asarray(E, f32)
    x = np.asarray(x).astype(np.int64).reshape(B * T)

    zproj = np.asarray(z0, f32) @ np.asarray(W_z0, f32).T  # (B, D)

    def t(a):
        return np.ascontiguousarray(np.asarray(a, f32).T).astype(bf)

    wqTn = np.stack([t(np.asarray(Wq[l], f32) * np.asarray(q_norm_w[l], f32)[None, :])
                     for l in range(L)])
    wkTn = np.stack([t(np.asarray(Wk[l], f32) * np.asarray(k_norm_w[l], f32)[None, :])
                     for l in range(L)])
    wvTn = np.stack([t(Wv[l]) for l in range(L)])
    woTn = np.stack([t(Wo[l]) for l in range(L)])
    wgTn = np.stack([t(np.asarray(Wg[l], f32) * np.asarray(norm2_w[l], f32)[None, :])
                     for l in range(L)])
    wuTn = np.stack([t(np.asarray(Wu[l], f32) * np.asarray(norm2_w[l], f32)[None, :])
                     for l in range(L)])
    wdTn = np.stack([t(Wd[l]) for l in range(L)])
    patchTn = t(np.asarray(patch_W, f32) * np.asarray(final_norm_w, f32)[None, :])
    n1rep = np.stack([np.broadcast_to(np.asarray(norm1_w[l], f32), (128, D)).copy()
                      for l in range(L)])

    wembT = np.asarray(W_embed_up, f32).T  # (P, D)
    in_maps = []
    for c in range(N_CORES):
        rows = x[c * R:(c + 1) * R]
        ex = E[rows]                       # (R, P)
        ex_augT = np.zeros((384, R), f32)
        ex_augT[:P, :] = ex.T
        ex_augT[P, :] = 1.0
        w_emb = np.zeros((384, D), f32)
        w_emb[:P, :] = wembT
        w_emb[P, :] = zproj[c // 4]

        qoff = (c % 4) * R
        kidx = np.arange(NKT * 128)
        qidx = qoff + np.arange(R)
        m = (kidx[:, None] <= qidx[None, :]).astype(f32)   # (1024, R)
        mask2 = np.broadcast_to(
            m.reshape(NKT, 128, 1, R), (NKT, 128, 2, R)).astype(bf).copy()

        ecTn = np.ascontiguousarray(E[c * VSH:(c + 1) * VSH].T).astype(bf)

        in_maps.append({
            "ex_augT": ex_augT.astype(bf), "w_emb": w_emb.astype(bf),
            "wqT": wqTn, "wkT": wkTn, "wvT": wvTn, "woT": woTn,
            "wgT": wgTn, "wuT": wuTn, "wdT": wdTn,
            "n1rep": n1rep, "mask2": mask2, "patchT": patchTn, "ecT": ecTn,
        })
    return in_maps


last_exec_ns = None


def kernel(**inputs) -> np.ndarray:
    global last_exec_ns
    in_maps = _prep_inputs(**inputs)
    nc = build()
    trace = bool(int(os.environ.get("TRN_PROFILE", "0")))
    kw = {}
    if trace:
        try:
            import prof_shim
            prof_shim.install()
            kw = dict(trace=True, tmpdir=os.environ.get("TRN_TRACE_DIR", None))
        except Exception:
            kw = {}
    res = run_bass_kernel_spmd(nc, in_maps, CORE_IDS, **kw)
    last_exec_ns = res.exec_time_ns
    parts = [np.asarray(res.results[c]["logits"]).astype(np.float32)
             for c in range(N_CORES)]
    return np.concatenate(parts, axis=1).reshape(B, T, V)


# revision 33
# speedup vs baseline: 1.1984x; 1.0656x over previous
"""Trainium2 distributed kernel for nn_ARDecoder (2x1024 tokens, D=1024,
H=16/KV=4 GQA, DFF=4096, V=32000, P=256, 4 layers).

Strategy: data-parallel over the 2048 (batch*seq) rows -- 256 rows per core.
Weights are replicated (bf16), activations stay SBUF-resident. Causal
attention uses a per-batch AllGather of K^T and V (replica groups
[[0..3],[4..7]]). The logits GEMM is vocab-sharded: h@patch_W.T is
all-gathered (tiny) and each core computes its 4000-column slice of E.
Host does the embedding gather E[x], norm-weight folding, transposes to
K-major weight layouts, and the final concat over vocab shards.
"""

import os
import numpy as np
import ml_dtypes

import concourse.bass as bass
import concourse.bacc as bacc
import concourse.mybir as mybir
import concourse.tile as tile
from concourse.bass_utils import run_bass_kernel_spmd
from concourse.masks import make_identity

BF16 = mybir.dt.bfloat16
F32 = mybir.dt.float32
AF = mybir.ActivationFunctionType

N_CORES = 8
CORE_IDS = list(range(N_CORES))
B, T, D, H, KV, HD, DFF, V, P, DLAT, L = 2, 1024, 1024, 16, 4, 64, 4096, 32000, 256, 512, 4
EPS = 1e-6
R = 256            # rows per core
RT = 2             # row tiles of 128
DKT = D // 128     # 8 k-tiles over D
VSH = V // N_CORES # 4000 vocab columns per core
NKT = 8            # key tiles of 128 within a batch

_cache = {}


def build(dbg=False):
    key = ("nc", dbg)
    if key in _cache:
        return _cache[key]
    nc = bacc.Bacc("TRN2", target_bir_lowering=False, debug=False,
                   num_devices=N_CORES)
    dbg_t = {}
    if dbg:
        for name, shape, dt in [
            ("dbg_h0", [128, RT, D], F32), ("dbg_h1", [128, RT, D], F32),
            ("dbg_h2", [128, RT, D], F32), ("dbg_h3", [128, RT, D], F32),
            ("dbg_h4", [128, RT, D], F32),
            ("dbg_qT", [128, DKT, R], BF16), ("dbg_kT", [128, KV, T], BF16),
            ("dbg_v65", [128, NKT, KV, HD + 1], BF16),
            ("dbg_oT", [128, 8, R], BF16), ("dbg_hp", [128, 2, R], BF16),
        ]:
            dbg_t[name] = nc.dram_tensor(name, shape, dt, kind="ExternalOutput")

    # ---- parameters (per-core inputs) ----
    ex_augT = nc.dram_tensor("ex_augT", [384, R], BF16, kind="ExternalInput")
    w_emb = nc.dram_tensor("w_emb", [384, D], BF16, kind="ExternalInput")
    wqT = nc.dram_tensor("wqT", [L, D, D], BF16, kind="ExternalInput")
    wkT = nc.dram_tensor("wkT", [L, D, KV * HD], BF16, kind="ExternalInput")
    wvT = nc.dram_tensor("wvT", [L, D, KV * HD], BF16, kind="ExternalInput")
    woT = nc.dram_tensor("woT", [L, D, D], BF16, kind="ExternalInput")
    wgT = nc.dram_tensor("wgT", [L, D, DFF], BF16, kind="ExternalInput")
    wuT = nc.dram_tensor("wuT", [L, D, DFF], BF16, kind="ExternalInput")
    wdT = nc.dram_tensor("wdT", [L, DFF, D], BF16, kind="ExternalInput")
    n1rep = nc.dram_tensor("n1rep", [L, 128, D], F32, kind="ExternalInput")
    mask2 = nc.dram_tensor("mask2", [NKT, 128, 2, R], BF16, kind="ExternalInput")
    patchT = nc.dram_tensor("patchT", [D, P], BF16, kind="ExternalInput")
    ecT = nc.dram_tensor("ecT", [P, VSH], BF16, kind="ExternalInput")
    out = nc.dram_tensor("logits", [B * T, VSH], BF16, kind="ExternalOutput")

    # ---- internal DRAM (collective bounce buffers) ----
    k_in, k_g, v_in, v_g = [], [], [], []
    for l in range(L):
        k_in.append(nc.dram_tensor(f"k_in{l}", [64, KV, R], BF16))
        k_g.append(nc.dram_tensor(f"k_g{l}", [4, 64, KV, R], BF16))
        v_in.append(nc.dram_tensor(f"v_in{l}", [128, RT, KV, HD], BF16))
        v_g.append(nc.dram_tensor(f"v_g{l}", [4, 128, RT, KV, HD], BF16))
    hp_in = nc.dram_tensor("hp_in", [128, 2, R], BF16)
    hp_g = nc.dram_tensor("hp_g", [8, 128, 2, R], BF16, addr_space="Shared")

    GROUPS_KV = [[0, 1, 2, 3], [4, 5, 6, 7]]
    GROUPS_ALL = [CORE_IDS]

    with tile.TileContext(nc) as tc:
        with (
            tc.tile_pool(name="const", bufs=1) as cpool,
            tc.tile_pool(name="persist", bufs=1) as pp,
            tc.tile_pool(name="wts", bufs=2) as wp,
            tc.tile_pool(name="acts", bufs=1) as ap,
        ):
            ident = cpool.tile([128, 128], BF16)
            make_identity(nc, ident[:])
            epsb = cpool.tile([128, 1], F32)
            nc.vector.memset(epsb[:], EPS)

            h = pp.tile([128, RT, D], F32)
            mask_sb = pp.tile([128, NKT, 2, R], BF16)
            nc.sync.dma_start(mask_sb[:], mask2[:, :, :, :])

            def rms_scales(src, tag):
                """Returns (r, s) tiles [128, RT] f32: r = 1/sqrt(ms+eps),
                s = sqrt(ms+eps)."""
                ss = ap.tile([128, RT], F32, name=f"ss_{tag}", tag=f"ss_{tag}")
                for rt in range(RT):
                    sq = ap.tile([128, D], F32, name=f"sq_{tag}{rt}",
                                 tag="sq_scratch", bufs=2)
                    nc.scalar.activation(sq[:], src[:, rt, :], AF.Square,
                                         accum_out=ss[:, rt : rt + 1])
                sg = ap.tile([128, RT], F32, name=f"sg_{tag}", tag=f"sg_{tag}")
                rr = ap.tile([128, RT], F32, name=f"rr_{tag}", tag=f"rr_{tag}")
                for rt in range(RT):  # per-rt so downstream rt0 work can start
                    nc.scalar.activation(sg[:, rt : rt + 1], ss[:, rt : rt + 1],
                                         AF.Sqrt, scale=1.0 / D, bias=epsb[:])
                    nc.vector.reciprocal_approx_fast(rr[:, rt : rt + 1],
                                                     sg[:, rt : rt + 1])
                return rr, sg

            def transpose_to(dst_ap, src_ap, psum_pool, tag):
                """PE-transpose one [128,128] bf16 tile src_ap -> dst_ap."""
                tp = psum_pool.tile([128, 128], BF16, name=f"tp_{tag}", tag="tp",
                                    bufs=2)
                nc.tensor.transpose(tp[:], src_ap, ident[:])
                nc.vector.tensor_copy(dst_ap, tp[:])

            # ================= embedding =================
            exT = ap.tile([128, 3, R], BF16)
            wem = ap.tile([128, 3, D], BF16)
            nc.sync.dma_start(exT[:], ex_augT[:].rearrange("(j p) d -> p j d", p=128))
            nc.sync.dma_start(wem[:], w_emb[:].rearrange("(j p) d -> p j d", p=128))
            with tc.tile_pool(name="ps_emb", bufs=4, space="PSUM") as pse:
                for rt in range(RT):
                    for nch in range(2):
                        psum = pse.tile([128, 512], F32, tag="mm")
                        for j in range(3):
                            nc.tensor.matmul(
                                psum[:],
                                exT[:, j, rt * 128:(rt + 1) * 128],
                                wem[:, j, nch * 512:(nch + 1) * 512],
                                start=(j == 0), stop=(j == 2))
                        nc.scalar.copy(h[:, rt, nch * 512:(nch + 1) * 512], psum[:])
            if dbg:
                nc.sync.dma_start(dbg_t["dbg_h0"][:], h[:])

            # ================= layers =================
            for l in range(L):
                # ---- norm1 + double-rms for q/k ----
                n1 = wp.tile([128, D], F32, tag="n1")
                nc.sync.dma_start(n1[:], n1rep[l])
                r1, _ = rms_scales(h, f"n1_{l}")
                # hw = h*norm1_w; ms(hn) = r1^2 * ms(hw), so the second rms
                # scale folds into the sqrt: qkb = hw * (r1*r2)
                hw = ap.tile([128, RT, D], F32, name=f"hw{l}", tag="hw_f32")
                ss2 = ap.tile([128, RT], F32, name=f"ss2_{l}", tag="ss2")
                for rt in range(RT):
                    nc.vector.tensor_tensor(hw[:, rt, :], h[:, rt, :], n1[:],
                                            mybir.AluOpType.mult)
                    sq2 = ap.tile([128, D], F32, name=f"sq2_{l}{rt}",
                                  tag="sq_scratch", bufs=2)
                    nc.scalar.activation(sq2[:], hw[:, rt, :], AF.Square,
                                         accum_out=ss2[:, rt : rt + 1])
                r1d = ap.tile([128, RT], F32, name=f"r1d{l}", tag="r1d")
                nc.vector.tensor_tensor(r1d[:], r1[:], r1[:], mybir.AluOpType.mult)
                nc.vector.tensor_scalar(r1d[:], r1d[:], 1.0 / D, None,
                                        mybir.AluOpType.mult)
                s2 = ap.tile([128, RT], F32, name=f"s2_{l}", tag="s2t")
                r12 = ap.tile([128, RT], F32, name=f"r12_{l}", tag="r12")
                qkb = ap.tile([128, RT, D], BF16, tag="qkb")
                for rt in range(RT):
                    nc.scalar.activation(s2[:, rt : rt + 1], ss2[:, rt : rt + 1],
                                         AF.Sqrt, scale=r1d[:, rt : rt + 1],
                                         bias=epsb[:])
                    r2s = ap.tile([128, 1], F32, name=f"r2s{l}{rt}", tag="r2s",
                                  bufs=2)
                    nc.vector.reciprocal_approx_fast(r2s[:], s2[:, rt : rt + 1])
                    nc.vector.tensor_tensor(r12[:, rt : rt + 1],
                                            r1[:, rt : rt + 1], r2s[:],
                                            mybir.AluOpType.mult)
                    nc.vector.tensor_scalar(qkb[:, rt, :], hw[:, rt, :],
                                            r12[:, rt : rt + 1], None,
                                            mybir.AluOpType.mult)
                qkT = ap.tile([128, DKT, R], BF16, tag="xT")
                with tc.tile_pool(name=f"ps_tp{l}", bufs=2, space="PSUM") as ptp:
                    for rt in range(RT):
                        for kt in range(DKT):
                            transpose_to(qkT[:, kt, rt * 128:(rt + 1) * 128],
                                         qkb[:, rt, kt * 128:(kt + 1) * 128],
                                         ptp, f"qk{l}")

                # ---- K, V first (kick the all-gather early), then Q ----
                qT = ap.tile([128, DKT, R], BF16, tag="qT")
                k_stage = ap.tile([128, 2, R], BF16, tag="k_stage")
                v_stage = ap.tile([128, RT, KV, HD], BF16, tag="v_stage")
                wk_sb = ap.tile([128, DKT, KV * HD], BF16, name=f"wk{l}", tag="wk_sb")
                wv_sb = ap.tile([128, DKT, KV * HD], BF16, name=f"wv{l}", tag="wv_sb")
                nc.sync.dma_start(wk_sb[:],
                                  wkT[l].rearrange("(kt p) d -> p kt d", p=128))
                nc.sync.dma_start(wv_sb[:],
                                  wvT[l].rearrange("(kt p) d -> p kt d", p=128))
                with tc.tile_pool(name=f"ps_qkv{l}", bufs=4, space="PSUM") as pq:
                    for mt in range(2):    # k^T [kvdim, rows]
                        psk = pq.tile([128, R], F32, tag="mm")
                        for kt in range(DKT):
                            nc.tensor.matmul(psk[:],
                                             wk_sb[:, kt, mt * 128:(mt + 1) * 128],
                                             qkT[:, kt, :],
                                             start=(kt == 0), stop=(kt == DKT - 1))
                        nc.scalar.copy(k_stage[:, mt, :], psk[:])
                    for g in range(KV):
                        nc.sync.dma_start(
                            k_in[l][:, g, :],
                            k_stage[64 * (g % 2):64 * (g % 2) + 64, g // 2, :])
                    nc.gpsimd.collective_compute(
                        "AllGather", mybir.AluOpType.bypass,
                        replica_groups=GROUPS_KV,
                        ins=[k_in[l][:].opt()], outs=[k_g[l][:].opt()])
                    for rt in range(RT):   # v [rows, kvdim] (undo r2 via s2)
                        psv = pq.tile([128, KV * HD], F32, tag="mm")
                        for kt in range(DKT):
                            nc.tensor.matmul(psv[:],
                                             qkT[:, kt, rt * 128:(rt + 1) * 128],
                                             wv_sb[:, kt, :],
                                             start=(kt == 0), stop=(kt == DKT - 1))
                        nc.vector.tensor_scalar(
                            v_stage[:, rt, :, :],
                            psv[:].rearrange("p (a b) -> p a b", a=KV),
                            s2[:, rt : rt + 1], None, mybir.AluOpType.mult)
                    nc.sync.dma_start(v_in[l][:], v_stage[:])
                    nc.gpsimd.collective_compute(
                        "AllGather", mybir.AluOpType.bypass,
                        replica_groups=GROUPS_KV,
                        ins=[v_in[l][:].opt()], outs=[v_g[l][:].opt()])
                    # Q while the gathers are in flight
                    wqb = ap.tile([128, DKT, D], BF16, name=f"wq{l}", tag="wbig",
                                  bufs=2)
                    nc.sync.dma_start(wqb[:],
                                      wqT[l].rearrange("(kt p) d -> p kt d", p=128))
                    for mt in range(DKT):  # q^T [qdim, rows]
                        psq = pq.tile([128, R], F32, tag="mm")
                        for kt in range(DKT):
                            nc.tensor.matmul(psq[:],
                                             wqb[:, kt, mt * 128:(mt + 1) * 128],
                                             qkT[:, kt, :],
                                             start=(kt == 0), stop=(kt == DKT - 1))
                        nc.scalar.copy(qT[:, mt, :], psq[:])

                kT_both = ap.tile([128, KV, T], BF16, tag="kT_both")
                v65 = ap.tile([128, NKT, KV, HD + 1], BF16, tag="v65")
                nc.vector.memset(v65[:], 1.0)
                for b in range(4):
                    nc.sync.dma_start(kT_both[0:64, :, b * R:(b + 1) * R],
                                      k_g[l][b])
                    nc.sync.dma_start(kT_both[64:128, :, b * R:(b + 1) * R],
                                      k_g[l][b])
                    nc.sync.dma_start(v65[:, 2 * b:2 * b + 2, :, 0:HD], v_g[l][b])

                if dbg and l == 0:
                    nc.sync.dma_start(dbg_t["dbg_qT"][:], qT[:])
                    nc.sync.dma_start(dbg_t["dbg_kT"][:], kT_both[:])
                    nc.sync.dma_start(dbg_t["dbg_v65"][:], v65[:])

                # ---- attention ----
                oT = ap.tile([128, 8, R], BF16, tag="oT")
                with (
                    tc.tile_pool(name=f"ps_sc{l}", bufs=2, space="PSUM") as psc,
                    tc.tile_pool(name=f"ps_ot{l}", bufs=1, space="PSUM") as pso,
                ):
                    for g in range(KV):
                        po = [pso.tile([HD + 1, R], F32, name=f"po{g}{s}",
                                       tag=f"ot{s}") for s in range(4)]
                        for kt in range(NKT):
                            # scores^T for all 4 q-heads of group g: region j
                            # is one psum bank; expM slot (j, i) = head 4g+j+2i
                            sc = psc.tile([128, 2, 2, R], F32, tag="sc")
                            for j in range(2):
                                nc.tensor.matmul(
                                    sc[:, j, :, :],
                                    kT_both[64 * j:64 * j + 64, g,
                                            kt * 128:(kt + 1) * 128],
                                    qT[64 * j:64 * j + 64, 2 * g:2 * g + 2, :],
                                    start=True, stop=True)
                            er = ap.tile([128, 2, 2, R], BF16, tag="expraw", bufs=4)
                            nc.scalar.activation(er[:], sc[:], AF.Exp,
                                                 scale=float(1.0 / np.sqrt(HD)))
                            expM = ap.tile([128, 2, 2, R], BF16, tag="expM", bufs=4)
                            nc.vector.tensor_tensor(
                                expM[:], er[:],
                                mask_sb[:, kt, 0:1, :].unsqueeze(2)
                                .broadcast_to((128, 2, 2, R)),
                                mybir.AluOpType.mult)
                            for s in range(4):
                                nc.tensor.matmul(
                                    po[s][:], v65[:, kt, g, :],
                                    expM[:, s % 2, s // 2, :],
                                    start=(kt == 0), stop=(kt == NKT - 1))
                        for s in range(4):
                            hq = 4 * g + s
                            den = ap.tile([1, R], F32, tag="den", bufs=2)
                            nc.scalar.copy(den[:], po[s][HD:HD + 1, :])
                            bcs = ap.tile([64, R], F32, tag="bcs", bufs=2)
                            nc.gpsimd.partition_broadcast(bcs[:], den[:])
                            rec = ap.tile([64, R], F32, tag="rec", bufs=2)
                            nc.vector.reciprocal_approx_fast(rec[:], bcs[:])
                            if hq % 2 == 0:
                                nc.vector.tensor_tensor(
                                    oT[0:64, hq // 2, :], po[s][0:HD, :], rec[:],
                                    mybir.AluOpType.mult)
                            else:
                                otmp = ap.tile([64, R], BF16, tag="otmp", bufs=2)
                                nc.vector.tensor_tensor(otmp[:], po[s][0:HD, :],
                                                        rec[:], mybir.AluOpType.mult)
                                nc.sync.dma_start(oT[64:128, hq // 2, :], otmp[:])

                if dbg and l == 0:
                    nc.sync.dma_start(dbg_t["dbg_oT"][:], oT[:])

                # ---- Wo + residual ----
                wob = ap.tile([128, DKT, D], BF16, name=f"wo{l}", tag="wbig",
                              bufs=2)
                nc.sync.dma_start(wob[:],
                                  woT[l].rearrange("(kt p) d -> p kt d", p=128))
                with tc.tile_pool(name=f"ps_wo{l}", bufs=1, space="PSUM") as pwo:
                    pswo = pwo.tile([128, RT, D], F32, tag="pswo")  # 4 banks
                    for kt in range(DKT):
                        for rt in range(RT):
                            for nch in range(2):
                                nc.tensor.matmul(
                                    pswo[:, rt, nch * 512:(nch + 1) * 512],
                                    oT[:, kt, rt * 128:(rt + 1) * 128],
                                    wob[:, kt, nch * 512:(nch + 1) * 512],
                                    start=(kt == 0), stop=(kt == DKT - 1))
                    for rt in range(RT):
                        nc.vector.tensor_tensor(h[:, rt, :], h[:, rt, :],
                                                pswo[:, rt, :], mybir.AluOpType.add)

                # ---- MLP (two DFF halves of 2048) ----
                r3, _ = rms_scales(h, f"n3_{l}")
                fnb = ap.tile([128, RT, D], BF16, tag="qkb")
                for rt in range(RT):
                    nc.vector.tensor_scalar(fnb[:, rt, :], h[:, rt, :],
                                            r3[:, rt : rt + 1], None,
                                            mybir.AluOpType.mult)
                fnT = ap.tile([128, DKT, R], BF16, tag="xT")
                with tc.tile_pool(name=f"ps_tpf{l}", bufs=2, space="PSUM") as ptf:
                    for rt in range(RT):
                        for kt in range(DKT):
                            transpose_to(fnT[:, kt, rt * 128:(rt + 1) * 128],
                                         fnb[:, rt, kt * 128:(kt + 1) * 128],
                                         ptf, f"fn{l}")

                for dh in range(2):
                    gs = ap.tile([128, RT, 2048], BF16, name=f"gs{l}{dh}", tag="gs")
                    us = ap.tile([128, RT, 2048], BF16, name=f"us{l}{dh}", tag="us")
                    for which, wsrc, dst in ((0, wgT, gs), (1, wuT, us)):
                        with tc.tile_pool(name=f"ps_ff{l}{dh}{which}", bufs=2,
                                          space="PSUM") as pff:
                            for grp in range(2):
                                psff = pff.tile([128, RT, 1024], F32, tag="psff")
                                col0 = dh * 2048 + grp * 1024
                                wfb = ap.tile([128, DKT, 1024], BF16,
                                              name=f"w{which}{l}{dh}{grp}",
                                              tag="wbig", bufs=2)
                                nc.sync.dma_start(
                                    wfb[:],
                                    wsrc[l].rearrange("(kt p) d -> p kt d",
                                                      p=128)[:, :, col0:col0 + 1024])
                                for kt in range(DKT):
                                    for rt in range(RT):
                                        for nch in range(2):
                                            nc.tensor.matmul(
                                                psff[:, rt,
                                                     nch * 512:(nch + 1) * 512],
                                                fnT[:, kt, rt * 128:(rt + 1) * 128],
                                                wfb[:, kt,
                                                    nch * 512:(nch + 1) * 512],
                                                start=(kt == 0),
                                                stop=(kt == DKT - 1))
                                for rt in range(RT):
                                    dap = dst[:, rt, grp * 1024:(grp + 1) * 1024]
                                    pap = psff[:, rt, :]
                                    if which == 0:
                                        nc.scalar.activation(dap, pap, AF.Silu)
                                    else:
                                        nc.vector.tensor_copy(dap, pap)
                    for rt in range(RT):
                        nc.vector.tensor_tensor(gs[:, rt, :], gs[:, rt, :],
                                                us[:, rt, :], mybir.AluOpType.mult)
                    mT = ap.tile([128, 16, R], BF16, name=f"mT{l}{dh}", tag="mT")
                    with tc.tile_pool(name=f"ps_tpm{l}{dh}", bufs=2,
                                      space="PSUM") as ptm:
                        for rt in range(RT):
                            for kt in range(16):
                                transpose_to(mT[:, kt, rt * 128:(rt + 1) * 128],
                                             gs[:, rt, kt * 128:(kt + 1) * 128],
                                             ptm, f"m{l}{dh}")
                    with tc.tile_pool(name=f"ps_wd{l}{dh}", bufs=1,
                                      space="PSUM") as pwd:
                        pswd = pwd.tile([128, RT, D], F32, tag="pswd")
                        wdr = wdT[l].rearrange("(kt p) d -> p kt d", p=128)
                        for blk in range(2):
                            wdb = ap.tile([128, DKT, D], BF16,
                                          name=f"wd{l}{dh}{blk}",
                                          tag="wbig", bufs=2)
                            nc.sync.dma_start(
                                wdb[:], wdr[:, dh * 16 + blk * 8:
                                            dh * 16 + (blk + 1) * 8, :])
                            for kt8 in range(DKT):
                                kt = blk * 8 + kt8
                                for rt in range(RT):
                                    for nch in range(2):
                                        nc.tensor.matmul(
                                            pswd[:, rt, nch * 512:(nch + 1) * 512],
                                            mT[:, kt, rt * 128:(rt + 1) * 128],
                                            wdb[:, kt8,
                                                nch * 512:(nch + 1) * 512],
                                            start=(kt == 0), stop=(kt == 15))
                        for rt in range(RT):
                            nc.vector.tensor_tensor(h[:, rt, :], h[:, rt, :],
                                                    pswd[:, rt, :],
                                                    mybir.AluOpType.add)
                if dbg:
                    nc.sync.dma_start(dbg_t[f"dbg_h{l + 1}"][:], h[:])

            # ================= final norm + patch + logits =================
            r4, _ = rms_scales(h, "fin")
            hfb = ap.tile([128, RT, D], BF16, tag="hnb")
            for rt in range(RT):
                nc.vector.tensor_scalar(hfb[:, rt, :], h[:, rt, :],
                                        r4[:, rt : rt + 1], None,
                                        mybir.AluOpType.mult)
            hfT = ap.tile([128, DKT, R], BF16, tag="xT")
            with tc.tile_pool(name="ps_tph", bufs=2, space="PSUM") as pth:
                for rt in range(RT):
                    for kt in range(DKT):
                        transpose_to(hfT[:, kt, rt * 128:(rt + 1) * 128],
                                     hfb[:, rt, kt * 128:(kt + 1) * 128],
                                     pth, "hf")
            pt_sb = wp.tile([128, DKT, P], BF16, tag="pt_sb", bufs=1)
            for kt in range(DKT):
                nc.sync.dma_start(pt_sb[:, kt, :], patchT[kt * 128:(kt + 1) * 128, :])
            hp_stage = ap.tile([128, 2, R], BF16, tag="hp_stage")
            with tc.tile_pool(name="ps_hp", bufs=2, space="PSUM") as php:
                for mt in range(2):
                    psp = php.tile([128, R], F32, tag="mm")
                    for kt in range(DKT):
                        nc.tensor.matmul(psp[:], pt_sb[:, kt, mt * 128:(mt + 1) * 128],
                                         hfT[:, kt, :],
                                         start=(kt == 0), stop=(kt == DKT - 1))
                    nc.scalar.copy(hp_stage[:, mt, :], psp[:])
            if dbg:
                nc.sync.dma_start(dbg_t["dbg_hp"][:], hp_stage[:])
            nc.sync.dma_start(hp_in[:], hp_stage[:])
            nc.gpsimd.collective_compute(
                "AllGather", mybir.AluOpType.bypass, replica_groups=GROUPS_ALL,
                ins=[hp_in[:].opt()], outs=[hp_g[:].opt()])
            hpT = ap.tile([128, 16, R], BF16, tag="mT")
            for rbk in range(8):
                nc.sync.dma_start(hpT[:, 2 * rbk:2 * rbk + 2, :], hp_g[rbk])
            ec0 = ap.tile([128, VSH], BF16, tag="gs")
            ec1 = ap.tile([128, VSH], BF16, tag="us")
            nc.sync.dma_start(ec0[:], ecT[0:128, :])
            nc.sync.dma_start(ec1[:], ecT[128:256, :])
            ec = [ec0, ec1]
            with tc.tile_pool(name="ps_lg", bufs=2, space="PSUM") as plg:
                for rbk in range(8):
                    for s in range(2):
                        for half in range(2):
                            plt = plg.tile([128, 4, 512], F32, tag="lg")
                            for kt in range(2):
                                for nq in range(4):
                                    nc.tensor.matmul(
                                        plt[:, nq, 0:500],
                                        hpT[:, 2 * rbk + kt, s * 128:(s + 1) * 128],
                                        ec[kt][:, half * 2000 + nq * 500:
                                               half * 2000 + (nq + 1) * 500],
                                        start=(kt == 0), stop=(kt == 1))
                            lg_sb = ap.tile([128, 2000], BF16, tag="lg_sb", bufs=3)
                            for nq in range(4):
                                if half == 0:
                                    nc.scalar.copy(lg_sb[:, nq * 500:(nq + 1) * 500],
                                                   plt[:, nq, 0:500])
                                else:
                                    nc.vector.tensor_copy(
                                        lg_sb[:, nq * 500:(nq + 1) * 500],
                                        plt[:, nq, 0:500])
                            nc.sync.dma_start(
                                out[rbk * 256 + s * 128: rbk * 256 + (s + 1) * 128,
                                    half * 2000:(half + 1) * 2000], lg_sb[:])

    nc.compile()
    _cache[key] = nc
    return nc


def _prep_inputs(x, z0, E, W_embed_up, W_z0, patch_W, final_norm_w,
                 norm1_w, q_norm_w, k_norm_w, norm2_w,
                 Wq, Wk, Wv, Wo, Wg, Wu, Wd):
    bf = ml_dtypes.bfloat16
    f32 = np.float32
    E = np.asarray(E, f32)
    x = np.asarray(x).astype(np.int64).reshape(B * T)

    zproj = np.asarray(z0, f32) @ np.asarray(W_z0, f32).T  # (B, D)

    def t(a):
        return np.ascontiguousarray(np.asarray(a, f32).T).astype(bf)

    wqTn = np.stack([t(np.asarray(Wq[l], f32) * np.asarray(q_norm_w[l], f32)[None, :])
                     for l in range(L)])
    wkTn = np.stack([t(np.asarray(Wk[l], f32) * np.asarray(k_norm_w[l], f32)[None, :])
                     for l in range(L)])
    wvTn = np.stack([t(Wv[l]) for l in range(L)])
    woTn = np.stack([t(Wo[l]) for l in range(L)])
    wgTn = np.stack([t(np.asarray(Wg[l], f32) * np.asarray(norm2_w[l], f32)[None, :])
                     for l in range(L)])
    wuTn = np.stack([t(np.asarray(Wu[l], f32) * np.asarray(norm2_w[l], f32)[None, :])
                     for l in range(L)])
    wdTn = np.stack([t(Wd[l]) for l in range(L)])
    patchTn = t(np.asarray(patch_W, f32) * np.asarray(final_norm_w, f32)[None, :])
    n1rep = np.stack([np.broadcast_to(np.asarray(norm1_w[l], f32), (128, D)).copy()
                      for l in range(L)])

    wembT = np.asarray(W_embed_up, f32).T  # (P, D)
    in_maps = []
    for c in range(N_CORES):
        rows = x[c * R:(c + 1) * R]
        ex = E[rows]                       # (R, P)
        ex_augT = np.zeros((384, R), f32)
        ex_augT[:P, :] = ex.T
        ex_augT[P, :] = 1.0
        w_emb = np.zeros((384, D), f32)
        w_emb[:P, :] = wembT
        w_emb[P, :] = zproj[c // 4]

        qoff = (c % 4) * R
        kidx = np.arange(NKT * 128)
        qidx = qoff + np.arange(R)
        m = (kidx[:, None] <= qidx[None, :]).astype(f32)   # (1024, R)
        mask2 = np.broadcast_to(
            m.reshape(NKT, 128, 1, R), (NKT, 128, 2, R)).astype(bf).copy()

        ecTn = np.ascontiguousarray(E[c * VSH:(c + 1) * VSH].T).astype(bf)

        in_maps.append({
            "ex_augT": ex_augT.astype(bf), "w_emb": w_emb.astype(bf),
            "wqT": wqTn, "wkT": wkTn, "wvT": wvTn, "woT": woTn,
            "wgT": wgTn, "wuT": wuTn, "wdT": wdTn,
            "n1rep": n1rep, "mask2": mask2, "patchT": patchTn, "ecT": ecTn,
        })
    return in_maps


last_exec_ns = None


def kernel(**inputs) -> np.ndarray:
    global last_exec_ns
    in_maps = _prep_inputs(**inputs)
    nc = build()
    trace = bool(int(os.environ.get("TRN_PROFILE", "0")))
    kw = {}
    if trace:
        try:
            import prof_shim
            prof_shim.install()
            kw = dict(trace=True, tmpdir=os.environ.get("TRN_TRACE_DIR", None))
        except Exception:
            kw = {}
    res = run_bass_kernel_spmd(nc, in_maps, CORE_IDS, **kw)
    last_exec_ns = res.exec_time_ns
    parts = [np.asarray(res.results[c]["logits"]).astype(np.float32)
             for c in range(N_CORES)]
    return np.concatenate(parts, axis=1).reshape(B, T, V)


# revision 37
# speedup vs baseline: 1.3141x; 1.0966x over previous
"""Trainium2 distributed kernel for nn_ARDecoder (2x1024 tokens, D=1024,
H=16/KV=4 GQA, DFF=4096, V=32000, P=256, 4 layers).

Strategy: data-parallel over the 2048 (batch*seq) rows -- 256 rows per core.
Weights are replicated (bf16), activations stay SBUF-resident. Causal
attention uses a per-batch AllGather of K^T and V (replica groups
[[0..3],[4..7]]). The logits GEMM is vocab-sharded: h@patch_W.T is
all-gathered (tiny) and each core computes its 4000-column slice of E.
Host does the embedding gather E[x], norm-weight folding, transposes to
K-major weight layouts, and the final concat over vocab shards.
"""

import os
import numpy as np
import ml_dtypes

import concourse.bass as bass
import concourse.bacc as bacc
import concourse.mybir as mybir
import concourse.tile as tile
from concourse.bass_utils import run_bass_kernel_spmd
from concourse.masks import make_identity

BF16 = mybir.dt.bfloat16
F32 = mybir.dt.float32
AF = mybir.ActivationFunctionType

N_CORES = 8
CORE_IDS = list(range(N_CORES))
B, T, D, H, KV, HD, DFF, V, P, DLAT, L = 2, 1024, 1024, 16, 4, 64, 4096, 32000, 256, 512, 4
EPS = 1e-6
R = 256            # rows per core
RT = 2             # row tiles of 128
DKT = D // 128     # 8 k-tiles over D
VSH = V // N_CORES # 4000 vocab columns per core
NKT = 8            # key tiles of 128 within a batch

_cache = {}


def build(dbg=False):
    key = ("nc", dbg)
    if key in _cache:
        return _cache[key]
    nc = bacc.Bacc("TRN2", target_bir_lowering=False, debug=False,
                   num_devices=N_CORES)
    dbg_t = {}
    if dbg:
        for name, shape, dt in [
            ("dbg_h0", [128, RT, D], F32), ("dbg_h1", [128, RT, D], F32),
            ("dbg_h2", [128, RT, D], F32), ("dbg_h3", [128, RT, D], F32),
            ("dbg_h4", [128, RT, D], F32),
            ("dbg_qT", [128, DKT, R], BF16), ("dbg_kT", [128, KV, T], BF16),
            ("dbg_v65", [128, NKT, KV, HD + 1], BF16),
            ("dbg_oT", [128, 8, R], BF16), ("dbg_hp", [128, 2, R], BF16),
        ]:
            dbg_t[name] = nc.dram_tensor(name, shape, dt, kind="ExternalOutput")

    # ---- parameters (per-core inputs) ----
    ex_augT = nc.dram_tensor("ex_augT", [384, R], BF16, kind="ExternalInput")
    w_emb = nc.dram_tensor("w_emb", [384, D], BF16, kind="ExternalInput")
    wqT = nc.dram_tensor("wqT", [L, D, D], BF16, kind="ExternalInput")
    wkT = nc.dram_tensor("wkT", [L, D, KV * HD], BF16, kind="ExternalInput")
    wvT = nc.dram_tensor("wvT", [L, D, KV * HD], BF16, kind="ExternalInput")
    woT = nc.dram_tensor("woT", [L, D, D], BF16, kind="ExternalInput")
    wgT = nc.dram_tensor("wgT", [L, D, DFF], BF16, kind="ExternalInput")
    wuT = nc.dram_tensor("wuT", [L, D, DFF], BF16, kind="ExternalInput")
    wdT = nc.dram_tensor("wdT", [L, DFF, D], BF16, kind="ExternalInput")
    n1rep = nc.dram_tensor("n1rep", [L, 128, D], F32, kind="ExternalInput")
    mask2 = nc.dram_tensor("mask2", [NKT, 128, 2, R], BF16, kind="ExternalInput")
    patchT = nc.dram_tensor("patchT", [D, P], BF16, kind="ExternalInput")
    ecT = nc.dram_tensor("ecT", [P, VSH], BF16, kind="ExternalInput")
    out = nc.dram_tensor("logits", [B * T, VSH], BF16, kind="ExternalOutput")

    # ---- internal DRAM (collective bounce buffers) ----
    k_in, k_g, v_in, v_g = [], [], [], []
    for l in range(L):
        k_in.append(nc.dram_tensor(f"k_in{l}", [64, KV, R], BF16))
        k_g.append(nc.dram_tensor(f"k_g{l}", [4, 64, KV, R], BF16))
        v_in.append(nc.dram_tensor(f"v_in{l}", [128, RT, KV, HD], BF16))
        v_g.append(nc.dram_tensor(f"v_g{l}", [4, 128, RT, KV, HD], BF16))
    hp_in = nc.dram_tensor("hp_in", [128, 2, R], BF16)
    hp_g = nc.dram_tensor("hp_g", [8, 128, 2, R], BF16, addr_space="Shared")

    GROUPS_KV = [[0, 1, 2, 3], [4, 5, 6, 7]]
    GROUPS_ALL = [CORE_IDS]

    with tile.TileContext(nc) as tc:
        with (
            tc.tile_pool(name="const", bufs=1) as cpool,
            tc.tile_pool(name="persist", bufs=1) as pp,
            tc.tile_pool(name="wts", bufs=2) as wp,
            tc.tile_pool(name="acts", bufs=1) as ap,
        ):
            ident = cpool.tile([128, 128], BF16)
            make_identity(nc, ident[:])
            epsb = cpool.tile([128, 1], F32)
            nc.vector.memset(epsb[:], EPS)

            h = pp.tile([128, RT, D], F32)
            mask_sb = pp.tile([128, NKT, 2, R], BF16)

            def rms_scales(src, tag):
                """Returns (r, s) tiles [128, RT] f32: r = 1/sqrt(ms+eps),
                s = sqrt(ms+eps)."""
                ss = ap.tile([128, RT], F32, name=f"ss_{tag}", tag=f"ss_{tag}")
                for rt in range(RT):
                    sq = ap.tile([128, D], F32, name=f"sq_{tag}{rt}",
                                 tag="sq_scratch", bufs=2)
                    nc.scalar.activation(sq[:], src[:, rt, :], AF.Square,
                                         accum_out=ss[:, rt : rt + 1])
                sg = ap.tile([128, RT], F32, name=f"sg_{tag}", tag=f"sg_{tag}")
                rr = ap.tile([128, RT], F32, name=f"rr_{tag}", tag=f"rr_{tag}")
                for rt in range(RT):  # per-rt so downstream rt0 work can start
                    nc.scalar.activation(sg[:, rt : rt + 1], ss[:, rt : rt + 1],
                                         AF.Sqrt, scale=1.0 / D, bias=epsb[:])
                    nc.vector.reciprocal_approx_fast(rr[:, rt : rt + 1],
                                                     sg[:, rt : rt + 1])
                return rr, sg

            def transpose_to(dst_ap, src_ap, psum_pool, tag):
                """PE-transpose one [128,128] bf16 tile src_ap -> dst_ap."""
                tp = psum_pool.tile([128, 128], BF16, name=f"tp_{tag}", tag="tp",
                                    bufs=2)
                nc.tensor.transpose(tp[:], src_ap, ident[:])
                nc.vector.tensor_copy(dst_ap, tp[:])

            # ================= embedding =================
            exT = ap.tile([128, 3, R], BF16)
            wem = ap.tile([128, 3, D], BF16)
            nc.sync.dma_start(exT[:], ex_augT[:].rearrange("(j p) d -> p j d", p=128))
            nc.sync.dma_start(wem[:], w_emb[:].rearrange("(j p) d -> p j d", p=128))
            nc.sync.dma_start(mask_sb[:], mask2[:, :, :, :])
            with tc.tile_pool(name="ps_emb", bufs=4, space="PSUM") as pse:
                for rt in range(RT):
                    for nch in range(2):
                        psum = pse.tile([128, 512], F32, tag="mm")
                        for j in range(3):
                            nc.tensor.matmul(
                                psum[:],
                                exT[:, j, rt * 128:(rt + 1) * 128],
                                wem[:, j, nch * 512:(nch + 1) * 512],
                                start=(j == 0), stop=(j == 2))
                        nc.scalar.copy(h[:, rt, nch * 512:(nch + 1) * 512], psum[:])
            if dbg:
                nc.sync.dma_start(dbg_t["dbg_h0"][:], h[:])

            # ================= layers =================
            for l in range(L):
                # ---- norm1 + double-rms for q/k ----
                n1 = wp.tile([128, D], F32, tag="n1")
                nc.sync.dma_start(n1[:], n1rep[l])
                r1, _ = rms_scales(h, f"n1_{l}")
                # hw = h*norm1_w; ms(hn) = r1^2 * ms(hw), so the second rms
                # scale folds into the sqrt: qkb = hw * (r1*r2)
                hw = ap.tile([128, RT, D], F32, name=f"hw{l}", tag="hw_f32")
                ss2 = ap.tile([128, RT], F32, name=f"ss2_{l}", tag="ss2")
                for rt in range(RT):
                    nc.vector.tensor_tensor(hw[:, rt, :], h[:, rt, :], n1[:],
                                            mybir.AluOpType.mult)
                    sq2 = ap.tile([128, D], F32, name=f"sq2_{l}{rt}",
                                  tag="sq_scratch", bufs=2)
                    nc.scalar.activation(sq2[:], hw[:, rt, :], AF.Square,
                                         accum_out=ss2[:, rt : rt + 1])
                r1d = ap.tile([128, RT], F32, name=f"r1d{l}", tag="r1d")
                nc.vector.tensor_tensor(r1d[:], r1[:], r1[:], mybir.AluOpType.mult)
                nc.vector.tensor_scalar(r1d[:], r1d[:], 1.0 / D, None,
                                        mybir.AluOpType.mult)
                s2 = ap.tile([128, RT], F32, name=f"s2_{l}", tag="s2t")
                r12 = ap.tile([128, RT], F32, name=f"r12_{l}", tag="r12")
                qkb = ap.tile([128, RT, D], BF16, tag="qkb")
                for rt in range(RT):
                    nc.scalar.activation(s2[:, rt : rt + 1], ss2[:, rt : rt + 1],
                                         AF.Sqrt, scale=r1d[:, rt : rt + 1],
                                         bias=epsb[:])
                    r2s = ap.tile([128, 1], F32, name=f"r2s{l}{rt}", tag="r2s",
                                  bufs=2)
                    nc.vector.reciprocal_approx_fast(r2s[:], s2[:, rt : rt + 1])
                    nc.vector.tensor_tensor(r12[:, rt : rt + 1],
                                            r1[:, rt : rt + 1], r2s[:],
                                            mybir.AluOpType.mult)
                    nc.vector.tensor_scalar(qkb[:, rt, :], hw[:, rt, :],
                                            r12[:, rt : rt + 1], None,
                                            mybir.AluOpType.mult)
                qkT = ap.tile([128, DKT, R], BF16, tag="xT")
                with tc.tile_pool(name=f"ps_tp{l}", bufs=3, space="PSUM") as ptp:
                    for kt in range(DKT):  # kt-outer: K matmul kt can start early
                        for rt in range(RT):
                            transpose_to(qkT[:, kt, rt * 128:(rt + 1) * 128],
                                         qkb[:, rt, kt * 128:(kt + 1) * 128],
                                         ptp, f"qk{l}")

                # ---- K, V first (kick the all-gather early), then Q ----
                qT = ap.tile([128, DKT, R], BF16, tag="qT")
                k_stage = ap.tile([128, 2, R], BF16, tag="k_stage")
                v_stage = ap.tile([128, RT, KV, HD], BF16, tag="v_stage")
                wk_sb = ap.tile([128, DKT, KV * HD], BF16, name=f"wk{l}", tag="wk_sb")
                wv_sb = ap.tile([128, DKT, KV * HD], BF16, name=f"wv{l}", tag="wv_sb")
                nc.sync.dma_start(wk_sb[:],
                                  wkT[l].rearrange("(kt p) d -> p kt d", p=128))
                nc.sync.dma_start(wv_sb[:],
                                  wvT[l].rearrange("(kt p) d -> p kt d", p=128))
                with tc.tile_pool(name=f"ps_qkv{l}", bufs=4, space="PSUM") as pq:
                    for mt in range(2):    # k^T [kvdim, rows]
                        psk = pq.tile([128, R], F32, tag="mm")
                        for kt in range(DKT):
                            nc.tensor.matmul(psk[:],
                                             wk_sb[:, kt, mt * 128:(mt + 1) * 128],
                                             qkT[:, kt, :],
                                             start=(kt == 0), stop=(kt == DKT - 1))
                        nc.scalar.copy(k_stage[:, mt, :], psk[:])
                    for g in range(KV):
                        nc.sync.dma_start(
                            k_in[l][:, g, :],
                            k_stage[64 * (g % 2):64 * (g % 2) + 64, g // 2, :])
                    nc.gpsimd.collective_compute(
                        "AllGather", mybir.AluOpType.bypass,
                        replica_groups=GROUPS_KV,
                        ins=[k_in[l][:].opt()], outs=[k_g[l][:].opt()])
                    for rt in range(RT):   # v [rows, kvdim] (undo r2 via s2)
                        psv = pq.tile([128, KV * HD], F32, tag="mm")
                        for kt in range(DKT):
                            nc.tensor.matmul(psv[:],
                                             qkT[:, kt, rt * 128:(rt + 1) * 128],
                                             wv_sb[:, kt, :],
                                             start=(kt == 0), stop=(kt == DKT - 1))
                        nc.vector.tensor_scalar(
                            v_stage[:, rt, :, :],
                            psv[:].rearrange("p (a b) -> p a b", a=KV),
                            s2[:, rt : rt + 1], None, mybir.AluOpType.mult)
                    nc.sync.dma_start(v_in[l][:], v_stage[:])
                    nc.gpsimd.collective_compute(
                        "AllGather", mybir.AluOpType.bypass,
                        replica_groups=GROUPS_KV,
                        ins=[v_in[l][:].opt()], outs=[v_g[l][:].opt()])
                    # Q while the gathers are in flight
                    wqb = ap.tile([128, DKT, D], BF16, name=f"wq{l}", tag="wbig",
                                  bufs=2)
                    nc.sync.dma_start(wqb[:],
                                      wqT[l].rearrange("(kt p) d -> p kt d", p=128))
                    for mt in range(DKT):  # q^T [qdim, rows]
                        psq = pq.tile([128, R], F32, tag="mm")
                        for kt in range(DKT):
                            nc.tensor.matmul(psq[:],
                                             wqb[:, kt, mt * 128:(mt + 1) * 128],
                                             qkT[:, kt, :],
                                             start=(kt == 0), stop=(kt == DKT - 1))
                        nc.scalar.copy(qT[:, mt, :], psq[:])

                kT_both = ap.tile([128, KV, T], BF16, tag="kT_both")
                v65 = ap.tile([128, NKT, KV, HD + 1], BF16, tag="v65")
                nc.vector.memset(v65[:], 1.0)
                for b in range(4):
                    nc.sync.dma_start(kT_both[0:64, :, b * R:(b + 1) * R],
                                      k_g[l][b])
                    nc.sync.dma_start(kT_both[64:128, :, b * R:(b + 1) * R],
                                      k_g[l][b])
                    nc.sync.dma_start(v65[:, 2 * b:2 * b + 2, :, 0:HD], v_g[l][b])

                if dbg and l == 0:
                    nc.sync.dma_start(dbg_t["dbg_qT"][:], qT[:])
                    nc.sync.dma_start(dbg_t["dbg_kT"][:], kT_both[:])
                    nc.sync.dma_start(dbg_t["dbg_v65"][:], v65[:])

                # ---- attention ----
                oT = ap.tile([128, 8, R], BF16, tag="oT")
                with (
                    tc.tile_pool(name=f"ps_sc{l}", bufs=2, space="PSUM") as psc,
                    tc.tile_pool(name=f"ps_ot{l}", bufs=1, space="PSUM") as pso,
                ):
                    for g in range(KV):
                        # po[j] accumulates heads (4g+j, 4g+j+2) side by side
                        po = [pso.tile([HD + 1, 2, R], F32, name=f"po{g}{j}",
                                       tag=f"ot{j}") for j in range(2)]
                        for kt in range(NKT):
                            # scores^T for all 4 q-heads of group g: region j
                            # is one psum bank; expM slot (j, i) = head 4g+j+2i
                            sc = psc.tile([128, 2, 2, R], F32, tag="sc", bufs=3)
                            for j in range(2):
                                nc.tensor.matmul(
                                    sc[:, j, :, :],
                                    kT_both[64 * j:64 * j + 64, g,
                                            kt * 128:(kt + 1) * 128],
                                    qT[64 * j:64 * j + 64, 2 * g:2 * g + 2, :],
                                    start=True, stop=True)
                            er = ap.tile([128, 2, 2, R], BF16, tag="expraw", bufs=6)
                            nc.scalar.activation(er[:], sc[:], AF.Exp,
                                                 scale=float(1.0 / np.sqrt(HD)))
                            expM = ap.tile([128, 2, 2, R], BF16, tag="expM", bufs=6)
                            nc.vector.tensor_tensor(
                                expM[:], er[:],
                                mask_sb[:, kt, 0:1, :].unsqueeze(2)
                                .broadcast_to((128, 2, 2, R)),
                                mybir.AluOpType.mult)
                            for j in range(2):
                                nc.tensor.matmul(
                                    po[j][:], v65[:, kt, g, :],
                                    expM[:, j, :, :],
                                    start=(kt == 0), stop=(kt == NKT - 1))
                        for s in range(4):
                            hq = 4 * g + s
                            j, i = s % 2, s // 2
                            den = ap.tile([1, R], F32, tag="den", bufs=2)
                            nc.scalar.copy(den[:], po[j][HD:HD + 1, i, :])
                            bcs = ap.tile([64, R], F32, tag="bcs", bufs=2)
                            nc.gpsimd.partition_broadcast(bcs[:], den[:])
                            rec = ap.tile([64, R], F32, tag="rec", bufs=2)
                            nc.vector.reciprocal_approx_fast(rec[:], bcs[:])
                            if hq % 2 == 0:
                                nc.vector.tensor_tensor(
                                    oT[0:64, hq // 2, :], po[j][0:HD, i, :], rec[:],
                                    mybir.AluOpType.mult)
                            else:
                                otmp = ap.tile([64, R], BF16, tag="otmp", bufs=2)
                                nc.vector.tensor_tensor(otmp[:], po[j][0:HD, i, :],
                                                        rec[:], mybir.AluOpType.mult)
                                nc.sync.dma_start(oT[64:128, hq // 2, :], otmp[:])

                if dbg and l == 0:
                    nc.sync.dma_start(dbg_t["dbg_oT"][:], oT[:])

                # ---- Wo + residual ----
                wob = ap.tile([128, DKT, D], BF16, name=f"wo{l}", tag="wbig",
                              bufs=2)
                nc.sync.dma_start(wob[:],
                                  woT[l].rearrange("(kt p) d -> p kt d", p=128))
                with tc.tile_pool(name=f"ps_wo{l}", bufs=1, space="PSUM") as pwo:
                    pswo = pwo.tile([128, RT, D], F32, tag="pswo")  # 4 banks
                    for kt in range(DKT):
                        for rt in range(RT):
                            for nch in range(2):
                                nc.tensor.matmul(
                                    pswo[:, rt, nch * 512:(nch + 1) * 512],
                                    oT[:, kt, rt * 128:(rt + 1) * 128],
                                    wob[:, kt, nch * 512:(nch + 1) * 512],
                                    start=(kt == 0), stop=(kt == DKT - 1))
                    for rt in range(RT):
                        nc.vector.tensor_tensor(h[:, rt, :], h[:, rt, :],
                                                pswo[:, rt, :], mybir.AluOpType.add)

                # ---- MLP (two DFF halves of 2048) ----
                r3, _ = rms_scales(h, f"n3_{l}")
                fnb = ap.tile([128, RT, D], BF16, tag="qkb")
                for rt in range(RT):
                    nc.vector.tensor_scalar(fnb[:, rt, :], h[:, rt, :],
                                            r3[:, rt : rt + 1], None,
                                            mybir.AluOpType.mult)
                fnT = ap.tile([128, DKT, R], BF16, tag="xT")
                with tc.tile_pool(name=f"ps_tpf{l}", bufs=2, space="PSUM") as ptf:
                    for rt in range(RT):
                        for kt in range(DKT):
                            transpose_to(fnT[:, kt, rt * 128:(rt + 1) * 128],
                                         fnb[:, rt, kt * 128:(kt + 1) * 128],
                                         ptf, f"fn{l}")

                for dh in range(2):
                    gs = ap.tile([128, RT, 2048], BF16, name=f"gs{l}{dh}", tag="gs")
                    us = ap.tile([128, RT, 2048], BF16, name=f"us{l}{dh}", tag="us")
                    for which, wsrc, dst in ((0, wgT, gs), (1, wuT, us)):
                        with tc.tile_pool(name=f"ps_ff{l}{dh}{which}", bufs=2,
                                          space="PSUM") as pff:
                            for grp in range(2):
                                psff = pff.tile([128, RT, 1024], F32, tag="psff")
                                col0 = dh * 2048 + grp * 1024
                                wfb = ap.tile([128, DKT, 1024], BF16,
                                              name=f"w{which}{l}{dh}{grp}",
                                              tag="wbig", bufs=2)
                                nc.sync.dma_start(
                                    wfb[:],
                                    wsrc[l].rearrange("(kt p) d -> p kt d",
                                                      p=128)[:, :, col0:col0 + 1024])
                                for kt in range(DKT):
                                    for rt in range(RT):
                                        for nch in range(2):
                                            nc.tensor.matmul(
                                                psff[:, rt,
                                                     nch * 512:(nch + 1) * 512],
                                                fnT[:, kt, rt * 128:(rt + 1) * 128],
                                                wfb[:, kt,
                                                    nch * 512:(nch + 1) * 512],
                                                start=(kt == 0),
                                                stop=(kt == DKT - 1))
                                for rt in range(RT):
                                    dap = dst[:, rt, grp * 1024:(grp + 1) * 1024]
                                    pap = psff[:, rt, :]
                                    if which == 0:
                                        nc.scalar.activation(dap, pap, AF.Silu)
                                    else:
                                        nc.vector.tensor_copy(dap, pap)
                    for rt in range(RT):
                        nc.vector.tensor_tensor(gs[:, rt, :], gs[:, rt, :],
                                                us[:, rt, :], mybir.AluOpType.mult)
                    mT = ap.tile([128, 16, R], BF16, name=f"mT{l}{dh}", tag="mT")
                    with tc.tile_pool(name=f"ps_tpm{l}{dh}", bufs=2,
                                      space="PSUM") as ptm:
                        for rt in range(RT):
                            for kt in range(16):
                                transpose_to(mT[:, kt, rt * 128:(rt + 1) * 128],
                                             gs[:, rt, kt * 128:(kt + 1) * 128],
                                             ptm, f"m{l}{dh}")
                    with tc.tile_pool(name=f"ps_wd{l}{dh}", bufs=1,
                                      space="PSUM") as pwd:
                        pswd = pwd.tile([128, RT, D], F32, tag="pswd")
                        wdr = wdT[l].rearrange("(kt p) d -> p kt d", p=128)
                        for blk in range(2):
                            wdb = ap.tile([128, DKT, D], BF16,
                                          name=f"wd{l}{dh}{blk}",
                                          tag="wbig", bufs=2)
                            nc.sync.dma_start(
                                wdb[:], wdr[:, dh * 16 + blk * 8:
                                            dh * 16 + (blk + 1) * 8, :])
                            for kt8 in range(DKT):
                                kt = blk * 8 + kt8
                                for rt in range(RT):
                                    for nch in range(2):
                                        nc.tensor.matmul(
                                            pswd[:, rt, nch * 512:(nch + 1) * 512],
                                            mT[:, kt, rt * 128:(rt + 1) * 128],
                                            wdb[:, kt8,
                                                nch * 512:(nch + 1) * 512],
                                            start=(kt == 0), stop=(kt == 15))
                        for rt in range(RT):
                            nc.vector.tensor_tensor(h[:, rt, :], h[:, rt, :],
                                                    pswd[:, rt, :],
                                                    mybir.AluOpType.add)
                if dbg:
                    nc.sync.dma_start(dbg_t[f"dbg_h{l + 1}"][:], h[:])

            # ================= final norm + patch + logits =================
            r4, _ = rms_scales(h, "fin")
            hfb = ap.tile([128, RT, D], BF16, tag="hnb")
            for rt in range(RT):
                nc.vector.tensor_scalar(hfb[:, rt, :], h[:, rt, :],
                                        r4[:, rt : rt + 1], None,
                                        mybir.AluOpType.mult)
            hfT = ap.tile([128, DKT, R], BF16, tag="xT")
            with tc.tile_pool(name="ps_tph", bufs=2, space="PSUM") as pth:
                for rt in range(RT):
                    for kt in range(DKT):
                        transpose_to(hfT[:, kt, rt * 128:(rt + 1) * 128],
                                     hfb[:, rt, kt * 128:(kt + 1) * 128],
                                     pth, "hf")
            pt_sb = wp.tile([128, DKT, P], BF16, tag="pt_sb", bufs=1)
            for kt in range(DKT):
                nc.sync.dma_start(pt_sb[:, kt, :], patchT[kt * 128:(kt + 1) * 128, :])
            hp_stage = ap.tile([128, 2, R], BF16, tag="hp_stage")
            with tc.tile_pool(name="ps_hp", bufs=2, space="PSUM") as php:
                for mt in range(2):
                    psp = php.tile([128, R], F32, tag="mm")
                    for kt in range(DKT):
                        nc.tensor.matmul(psp[:], pt_sb[:, kt, mt * 128:(mt + 1) * 128],
                                         hfT[:, kt, :],
                                         start=(kt == 0), stop=(kt == DKT - 1))
                    nc.scalar.copy(hp_stage[:, mt, :], psp[:])
            if dbg:
                nc.sync.dma_start(dbg_t["dbg_hp"][:], hp_stage[:])
            nc.sync.dma_start(hp_in[:], hp_stage[:])
            nc.gpsimd.collective_compute(
                "AllGather", mybir.AluOpType.bypass, replica_groups=GROUPS_ALL,
                ins=[hp_in[:].opt()], outs=[hp_g[:].opt()])
            hpT = ap.tile([128, 16, R], BF16, tag="mT")
            for rbk in range(8):
                nc.sync.dma_start(hpT[:, 2 * rbk:2 * rbk + 2, :], hp_g[rbk])
            ec0 = ap.tile([128, VSH], BF16, tag="gs")
            ec1 = ap.tile([128, VSH], BF16, tag="us")
            nc.sync.dma_start(ec0[:], ecT[0:128, :])
            nc.sync.dma_start(ec1[:], ecT[128:256, :])
            ec = [ec0, ec1]
            with tc.tile_pool(name="ps_lg", bufs=2, space="PSUM") as plg:
                for rbk in range(8):
                    for s in range(2):
                        for half in range(2):
                            plt = plg.tile([128, 4, 512], F32, tag="lg")
                            for kt in range(2):
                                for nq in range(4):
                                    nc.tensor.matmul(
                                        plt[:, nq, 0:500],
                                        hpT[:, 2 * rbk + kt, s * 128:(s + 1) * 128],
                                        ec[kt][:, half * 2000 + nq * 500:
                                               half * 2000 + (nq + 1) * 500],
                                        start=(kt == 0), stop=(kt == 1))
                            lg_sb = ap.tile([128, 2000], BF16, tag="lg_sb", bufs=3)
                            for nq in range(4):
                                if half == 0:
                                    nc.scalar.copy(lg_sb[:, nq * 500:(nq + 1) * 500],
                                                   plt[:, nq, 0:500])
                                else:
                                    nc.vector.tensor_copy(
                                        lg_sb[:, nq * 500:(nq + 1) * 500],
                                        plt[:, nq, 0:500])
                            nc.sync.dma_start(
                                out[rbk * 256 + s * 128: rbk * 256 + (s + 1) * 128,
                                    half * 2000:(half + 1) * 2000], lg_sb[:])

    nc.compile()
    _cache[key] = nc
    return nc


def _prep_inputs(x, z0, E, W_embed_up, W_z0, patch_W, final_norm_w,
                 norm1_w, q_norm_w, k_norm_w, norm2_w,
                 Wq, Wk, Wv, Wo, Wg, Wu, Wd):
    bf = ml_dtypes.bfloat16
    f32 = np.float32
    E = np.asarray(E, f32)
    x = np.asarray(x).astype(np.int64).reshape(B * T)

    zproj = np.asarray(z0, f32) @ np.asarray(W_z0, f32).T  # (B, D)

    def t(a):
        return np.ascontiguousarray(np.asarray(a, f32).T).astype(bf)

    wqTn = np.stack([t(np.asarray(Wq[l], f32) * np.asarray(q_norm_w[l], f32)[None, :])
                     for l in range(L)])
    wkTn = np.stack([t(np.asarray(Wk[l], f32) * np.asarray(k_norm_w[l], f32)[None, :])
                     for l in range(L)])
    wvTn = np.stack([t(Wv[l]) for l in range(L)])
    woTn = np.stack([t(Wo[l]) for l in range(L)])
    wgTn = np.stack([t(np.asarray(Wg[l], f32) * np.asarray(norm2_w[l], f32)[None, :])
                     for l in range(L)])
    wuTn = np.stack([t(np.asarray(Wu[l], f32) * np.asarray(norm2_w[l], f32)[None, :])
                     for l in range(L)])
    wdTn = np.stack([t(Wd[l]) for l in range(L)])
    patchTn = t(np.asarray(patch_W, f32) * np.asarray(final_norm_w, f32)[None, :])
    n1rep = np.stack([np.broadcast_to(np.asarray(norm1_w[l], f32), (128, D)).copy()
                      for l in range(L)])

    wembT = np.asarray(W_embed_up, f32).T  # (P, D)
    in_maps = []
    for c in range(N_CORES):
        rows = x[c * R:(c + 1) * R]
        ex = E[rows]                       # (R, P)
        ex_augT = np.zeros((384, R), f32)
        ex_augT[:P, :] = ex.T
        ex_augT[P, :] = 1.0
        w_emb = np.zeros((384, D), f32)
        w_emb[:P, :] = wembT
        w_emb[P, :] = zproj[c // 4]

        qoff = (c % 4) * R
        kidx = np.arange(NKT * 128)
        qidx = qoff + np.arange(R)
        m = (kidx[:, None] <= qidx[None, :]).astype(f32)   # (1024, R)
        mask2 = np.broadcast_to(
            m.reshape(NKT, 128, 1, R), (NKT, 128, 2, R)).astype(bf).copy()

        ecTn = np.ascontiguousarray(E[c * VSH:(c + 1) * VSH].T).astype(bf)

        in_maps.append({
            "ex_augT": ex_augT.astype(bf), "w_emb": w_emb.astype(bf),
            "wqT": wqTn, "wkT": wkTn, "wvT": wvTn, "woT": woTn,
            "wgT": wgTn, "wuT": wuTn, "wdT": wdTn,
            "n1rep": n1rep, "mask2": mask2, "patchT": patchTn, "ecT": ecTn,
        })
    return in_maps


last_exec_ns = None


def kernel(**inputs) -> np.ndarray:
    global last_exec_ns
    in_maps = _prep_inputs(**inputs)
    nc = build()
    trace = bool(int(os.environ.get("TRN_PROFILE", "0")))
    kw = {}
    if trace:
        try:
            import prof_shim
            prof_shim.install()
            kw = dict(trace=True, tmpdir=os.environ.get("TRN_TRACE_DIR", None))
        except Exception:
            kw = {}
    res = run_bass_kernel_spmd(nc, in_maps, CORE_IDS, **kw)
    last_exec_ns = res.exec_time_ns
    parts = [np.asarray(res.results[c]["logits"]).astype(np.float32)
             for c in range(N_CORES)]
    return np.concatenate(parts, axis=1).reshape(B, T, V)
